# revision 11
# baseline (speedup 1.0000x reference)
"""BotRGCN Trainium2 kernel: feature transform + 2 RGCN layers + classifier.

Sharding: nodes split across 8 cores by id (12500/core, padded to 12544).
Edges partitioned by destination shard; per (relation, dst-window, src-bank)
groups padded to a block structure uniform across cores so a single SPMD
program serves all 8 cores. Source features exchanged via bf16 AllGather of
the per-layer node-feature table; gathers via int16 dma_gather per src bank,
round-robined over 4 SWDGE queues so descriptor generation uses all Q7 core
pairs. The per-block scatter one-hot (edge -> dst column, mean weight folded
in) is precomputed on host and streamed from DRAM, keeping DVE off the
critical path.
"""

import sys

sys.path.insert(0, "/opt/trn_rl_repo")

from contextlib import ExitStack

import numpy as np
import ml_dtypes

import concourse.bass as bass
import concourse.bacc as bacc
import concourse.mybir as mybir
import concourse.tile as tile
from concourse.masks import make_identity
from concourse.bass_utils import run_bass_kernel_spmd

BF16 = mybir.dt.bfloat16
F32 = mybir.dt.float32
I16 = mybir.dt.int16

P = 128

# full-problem config (test.py overrides for mini runs)
CFG = dict(
    N=100000,        # nodes
    NC=8,            # cores
    R=2,             # relations
    H=128,
    DES=768, TWEET=768, NUMP=6, CATP=11,
    WIN=256,         # dst window (PSUM free dim)
    NBLK_CH=12,      # gather-chunk size in 128-edge blocks
    ST_CH=8,         # st-stream chunk size in blocks
    BANKROWS=25088,  # gather-table bank rows (< 2^15)
    NTF=512,         # feature-stage node tile
)


def _derived(cfg):
    d = dict(cfg)
    d["SH"] = cfg["N"] // cfg["NC"]
    d["SHP"] = ((d["SH"] + P - 1) // P) * P
    d["NW"] = d["SHP"] // cfg["WIN"]
    assert d["SHP"] % cfg["WIN"] == 0
    d["TROWS"] = cfg["NC"] * d["SHP"]           # padded table rows
    d["BANKS"] = (d["TROWS"] + cfg["BANKROWS"] - 1) // cfg["BANKROWS"]
    d["TBLK"] = d["SHP"] // P                   # 128-row blobs per core
    # x feature layout: [des | tweet | num(pad to 128) | cat(pad to 128)]
    d["KDES"] = cfg["DES"] // P
    d["KTWEET"] = cfg["TWEET"] // P
    d["KX"] = d["KDES"] + d["KTWEET"] + 2
    d["XROWS"] = d["KX"] * P
    return d


# ---------------------------------------------------------------------------
# host-side graph planning
# ---------------------------------------------------------------------------

class Plan:
    pass


def build_plan(edge_index, edge_type, cfg):
    """Group edges per core by (rel, dst-window, src-bank); pad each group to a
    whole number of 128-edge blocks, uniform across cores. Returns per-core
    gather-index arrays, the streamed scatter one-hot tiles (emission order),
    plus the uniform block structure."""
    d = cfg
    NC, SH, SHP, WIN, NW = d["NC"], d["SH"], d["SHP"], d["WIN"], d["NW"]
    BANKS, BR, NBLK_CH = d["BANKS"], d["BANKROWS"], d["NBLK_CH"]
    ST_CH = d["ST_CH"]
    R = d["R"]
    N = d["N"]
    TBLK = d["TBLK"]

    src = np.asarray(edge_index[0], dtype=np.int64)
    dst = np.asarray(edge_index[1], dtype=np.int64)
    et = np.asarray(edge_type, dtype=np.int64)

    core = dst // SH
    dl = dst - core * SH
    # table row of a (padded) node: blob layout [p][t] per core
    sl = src - (src // SH) * SH
    ps = (src // SH) * SHP + (sl % P) * TBLK + (sl // P)
    bank = ps // BR
    bidx = (ps - bank * BR).astype(np.int16)
    win = dl // WIN
    dw = (dl - win * WIN).astype(np.int64)

    # per-(rel, node) in-degree -> per-edge mean weight
    cnt = np.bincount(et * N + dst, minlength=R * N).reshape(R, N)
    wv = (1.0 / np.maximum(cnt, 1.0))[et, dst].astype(np.float32)

    # group = (rel, bank, win); uniform block counts = max over cores
    NG = R * BANKS * NW
    gid = (et * BANKS + bank) * NW + win
    counts = np.bincount(core * NG + gid, minlength=NC * NG).reshape(NC, NG)
    bpg = ((counts.max(axis=0) + P - 1) // P).reshape(R, BANKS, NW)
    # chunk padding: extra all-zero blocks at the end of each (rel, bank)
    # stream so streams are whole chunks; these are skipped in emission and
    # trimmed from the gather via trailing -1 indices.
    padblk = np.zeros((R, BANKS), np.int64)
    for r in range(R):
        for b in range(BANKS):
            tot = int(bpg[r, b].sum())
            pad = (-tot) % NBLK_CH
            if tot == 0 and pad == 0:
                pad = NBLK_CH
            padblk[r, b] = pad

    # stream layout (gather order): per (rel, bank): groups w=0..NW-1 then pad
    group_blk_base = np.zeros((R, BANKS, NW), np.int64)
    stream_blk_base = np.zeros((R, BANKS), np.int64)
    stream_nblk = np.zeros((R, BANKS), np.int64)
    base = 0
    for r in range(R):
        for b in range(BANKS):
            stream_blk_base[r, b] = base
            for w in range(NW):
                group_blk_base[r, b, w] = base
                base += int(bpg[r, b, w])
            base += int(padblk[r, b])
            stream_nblk[r, b] = base - stream_blk_base[r, b]
    TOTBLK = base
    TOTSLOT = TOTBLK * P

    # emission order (matmul/st-stream order): m over (w, r, b, k)
    EMIT_TOT = int(bpg.sum())
    EMIT_PAD = ((EMIT_TOT + ST_CH - 1) // ST_CH) * ST_CH
    m_of_blk = np.full(TOTBLK, -1, np.int64)
    m = 0
    for w in range(NW):
        for r in range(R):
            for b in range(BANKS):
                g0 = group_blk_base[r, b, w]
                for k in range(int(bpg[r, b, w])):
                    m_of_blk[g0 + k] = m
                    m += 1
    assert m == EMIT_TOT

    # place each edge into its group's slot range (per core)
    slots_per_group = np.zeros(NG, np.int64)
    for r in range(R):
        for b in range(BANKS):
            for w in range(NW):
                slots_per_group[(r * BANKS + b) * NW + w] = bpg[r, b, w] * P
    grp_slot_base = np.zeros(NG, np.int64)
    for r in range(R):
        for b in range(BANKS):
            for w in range(NW):
                grp_slot_base[(r * BANKS + b) * NW + w] = group_blk_base[r, b, w] * P

    okey = core * NG + gid
    order = np.argsort(okey, kind="stable")
    so = okey[order]
    first_of = np.r_[True, so[1:] != so[:-1]]
    idx_in_run = np.arange(len(so)) - np.maximum.accumulate(
        np.where(first_of, np.arange(len(so)), 0)
    )
    slot = grp_slot_base[so % NG] + idx_in_run

    idx16 = np.zeros((NC, 8 * 16, TOTSLOT // 16), np.int16)
    # trailing chunk-pad slots: -1 so the Q7 kernel drops their descriptors
    for r in range(R):
        for b in range(BANKS):
            pe = stream_blk_base[r, b] + stream_nblk[r, b]
            p0 = pe - padblk[r, b]
            if pe > p0:
                s0, s1 = p0 * P, pe * P
                for g in range(8):
                    idx16[:, 16 * g + (np.arange(s0, s1) % 16),
                          np.arange(s0, s1) // 16] = -1

    ecore = core[order]
    col = slot // 16
    prow = (slot % 16).astype(np.int64)
    for g in range(8):
        idx16[ecore, 16 * g + prow, col] = bidx[order]

    # streamed scatter tiles, emission order: stT[p, m, j]
    stT = np.zeros((NC, P, EMIT_PAD, WIN), ml_dtypes.bfloat16)
    stT[ecore, slot % P, m_of_blk[slot // P], dw[order]] = wv[order]

    pl = Plan()
    pl.idx16 = idx16.reshape(NC, P, TOTSLOT // 16)
    pl.stT = stT
    pl.bpg = bpg
    pl.padblk = padblk
    pl.TOTBLK = TOTBLK
    pl.EMIT_TOT = EMIT_TOT
    pl.EMIT_PAD = EMIT_PAD
    pl.group_blk_base = group_blk_base
    pl.stream_blk_base = stream_blk_base
    pl.stream_nblk = stream_nblk
    return pl


def prep_x(x, cfg):
    """Per-core transposed bf16 feature blocks [XROWS, SHP]."""
    d = cfg
    NC, SH, SHP = d["NC"], d["SH"], d["SHP"]
    NUMP, TWEET, CATP, DES = d["NUMP"], d["TWEET"], d["CATP"], d["DES"]
    KD, KT = d["KDES"], d["KTWEET"]
    out = np.zeros((NC, d["XROWS"], SHP), ml_dtypes.bfloat16)
    for c in range(NC):
        xs = x[c * SH:(c + 1) * SH]
        xT = np.zeros((d["XROWS"], SHP), np.float32)
        xT[:DES, :SH] = xs[:, NUMP + TWEET + CATP:].T
        xT[DES:DES + TWEET, :SH] = xs[:, NUMP:NUMP + TWEET].T
        xT[(KD + KT) * P:(KD + KT) * P + NUMP, :SH] = xs[:, :NUMP].T
        xT[(KD + KT + 1) * P:(KD + KT + 1) * P + CATP, :SH] = \
            xs[:, NUMP + TWEET:NUMP + TWEET + CATP].T
        out[c] = xT.astype(ml_dtypes.bfloat16)
    return out


def prep_weights(inp, cfg):
    """bf16 weight blocks + packed fp32 biases."""
    bf = lambda a: np.asarray(a, np.float32).astype(ml_dtypes.bfloat16)
    d = cfg
    wnum = np.zeros((P, d["H"]), np.float32)
    wnum[:d["NUMP"]] = inp["W_num"]
    wcat = np.zeros((P, d["H"]), np.float32)
    wcat[:d["CATP"]] = inp["W_cat"]
    w = {
        "wdes": bf(inp["W_des"]), "wtweet": bf(inp["W_tweet"]),
        "wnum": bf(wnum), "wcat": bf(wcat), "win": bf(inp["W_in"]),
        "root1": bf(inp["root1"]), "rel10": bf(inp["rel1"][0]),
        "rel11": bf(inp["rel1"][1]),
        "root2": bf(inp["root2"]), "rel20": bf(inp["rel2"][0]),
        "rel21": bf(inp["rel2"][1]), "wcls": bf(inp["W_cls"]),
    }
    biases = np.stack(
        [inp["b_des"], inp["b_tweet"], inp["b_num"], inp["b_cat"],
         inp["b_in"], inp["prelu_a"], inp["bias1"], inp["bias2"],
         inp["b_cls"]], axis=1).astype(np.float32)   # [128, 9]
    w["biases"] = biases
    return w


# ---------------------------------------------------------------------------
# bass program
# ---------------------------------------------------------------------------

def build_bass(cfg, pl):
    d = cfg
    NC, SHP, WIN, NW, NTF = d["NC"], d["SHP"], d["WIN"], d["NW"], d["NTF"]
    BANKS, BR, NBLK_CH = d["BANKS"], d["BANKROWS"], d["NBLK_CH"]
    ST_CH = d["ST_CH"]
    R, H = d["R"], d["H"]
    KD, KT, KX = d["KDES"], d["KTWEET"], d["KX"]
    TBLK = d["TBLK"]
    TROWS = d["TROWS"]
    CHS = NBLK_CH * P      # idx slots per chunk

    nc = bacc.Bacc(None, target_bir_lowering=False, debug=False,
                   num_devices=NC, num_swdge_queues=4,
                   dynamic_dma_scratch_size=65536)

    # ---- I/O ----
    xT = nc.dram_tensor("xT", [d["XROWS"], SHP], BF16, kind="ExternalInput")
    idxt = nc.dram_tensor("idxt", [P, pl.TOTBLK * P // 16], I16, kind="ExternalInput")
    stt = nc.dram_tensor("stt", [P, pl.EMIT_PAD, WIN], BF16, kind="ExternalInput")
    wts = {}
    for nm, shp in [("wdes", [d["DES"], H]), ("wtweet", [d["TWEET"], H]),
                    ("wnum", [P, H]), ("wcat", [P, H]), ("win", [4 * P, H]),
                    ("root1", [H, H]), ("rel10", [H, H]), ("rel11", [H, H]),
                    ("root2", [H, H]), ("rel20", [H, H]), ("rel21", [H, H]),
                    ("wcls", [H, H])]:
        wts[nm] = nc.dram_tensor(nm, shp, BF16, kind="ExternalInput")
    biases = nc.dram_tensor("biases", [P, 9], F32, kind="ExternalInput")
    outT = nc.dram_tensor("outT", [P, SHP], F32, kind="ExternalOutput")

    # ---- collective tables ----
    cc_in = [nc.dram_tensor(f"cc{i}_in", [SHP, H], BF16, kind="Internal")
             for i in (1, 2)]
    cc_out = [nc.dram_tensor(f"cc{i}_out", [NC * SHP, H], BF16,
                             kind="Internal", addr_space="Shared")
              for i in (1, 2)]

    rg = [list(range(NC))]

    with tile.TileContext(nc) as tc:
        with (
            tc.tile_pool(name="const", bufs=1) as cpool,
            tc.tile_pool(name="resident", bufs=1) as rpool,
            ExitStack() as mstack,
        ):
            # ---- constants ----
            ident = cpool.tile([P, P], BF16)
            make_identity(nc, ident[:])
            bias_t = cpool.tile([P, 9], F32)
            nc.sync.dma_start(out=bias_t[:], in_=biases[:])

            wt = {}
            for nm, kb in [("wdes", KD), ("wtweet", KT), ("wnum", 1),
                           ("wcat", 1), ("win", 4), ("root1", 1),
                           ("rel10", 1), ("rel11", 1), ("root2", 1),
                           ("rel20", 1), ("rel21", 1), ("wcls", 1)]:
                t = cpool.tile([P, kb, H], BF16, tag=f"w_{nm}", name=f"w_{nm}")
                nc.sync.dma_start(
                    out=t[:], in_=wts[nm].rearrange("(k p) h -> p k h", p=P))
                wt[nm] = t

            # resident activations (transposed, [H, SHP] bf16)
            hT = [rpool.tile([P, SHP], BF16, tag="ht", name=f"hT{i}", bufs=2)
                  for i in range(2)]

            # =============== feature transform ===============
            fstack = ExitStack()
            fpool = fstack.enter_context(tc.tile_pool(name="featsb", bufs=2))
            fpp = fstack.enter_context(
                tc.tile_pool(name="featps", bufs=2, space="PSUM"))
            ntiles = (SHP + NTF - 1) // NTF
            for t in range(ntiles):
                n0 = t * NTF
                n1 = min(SHP, n0 + NTF)
                nn = n1 - n0
                xt = fpool.tile([P, KX, NTF], BF16, tag="xt", name="xt")
                nc.sync.dma_start(
                    out=xt[:, :, :nn],
                    in_=xT.rearrange("(k p) n -> p k n", p=P)[:, :, n0:n1])

                zb = []
                for bi, (wnm, ks, kn) in enumerate([
                        ("wdes", 0, KD), ("wtweet", KD, KT),
                        ("wnum", KD + KT, 1), ("wcat", KD + KT + 1, 1)]):
                    pz = fpp.tile([P, NTF], F32, tag=f"pz{bi}", name=f"pz{bi}", space="PSUM", bufs=1)
                    for k in range(kn):
                        nc.tensor.matmul(
                            out=pz[:, :nn], lhsT=wt[wnm][:, k, :],
                            rhs=xt[:, ks + k, :nn],
                            start=(k == 0), stop=(k == kn - 1))
                    v = fpool.tile([P, NTF], BF16, tag=f"v{bi}", name=f"v{bi}")
                    nc.scalar.activation(
                        out=v[:, :nn], in_=pz[:, :nn],
                        func=mybir.ActivationFunctionType.Identity,
                        bias=bias_t[:, bi:bi + 1])
                    z = fpool.tile([P, NTF], BF16, tag=f"z{bi}", name=f"z{bi}")
                    nc.vector.scalar_tensor_tensor(
                        out=z[:, :nn], in0=v[:, :nn], scalar=0.01,
                        in1=v[:, :nn], op0=mybir.AluOpType.mult,
                        op1=mybir.AluOpType.max)
                    zb.append(z)

                ph = fpp.tile([P, NTF], F32, tag="ph", name="ph", space="PSUM")
                for k in range(4):
                    nc.tensor.matmul(out=ph[:, :nn], lhsT=wt["win"][:, k, :],
                                     rhs=zb[k][:, :nn],
                                     start=(k == 0), stop=(k == 3))
                vh = fpool.tile([P, NTF], F32, tag="vh", name="vh")
                nc.scalar.activation(
                    out=vh[:, :nn], in_=ph[:, :nn],
                    func=mybir.ActivationFunctionType.Identity,
                    bias=bias_t[:, 4:5])
                nc.vector.scalar_tensor_tensor(
                    out=hT[0][:, n0:n1], in0=vh[:, :nn],
                    scalar=bias_t[:, 5:6], in1=vh[:, :nn],
                    op0=mybir.AluOpType.mult, op1=mybir.AluOpType.max)

            fstack.close()
            wpool = mstack.enter_context(tc.tile_pool(name="work", bufs=3))
            ppool = mstack.enter_context(
                tc.tile_pool(name="psum", bufs=2, space="PSUM"))

            # =============== per-layer helper ===============
            def emit_table(src_hT, cc_in_t, cc_out_t):
                RB = 7   # transpose blocks per staging tile / DMA
                ccv = cc_in_t.rearrange("(p t) h -> p t h", p=P)
                for b0 in range(0, TBLK, RB):
                    rows = wpool.tile([P, RB, P], BF16, tag="rows",
                                      name="rows", bufs=2)
                    for blk in range(b0, min(b0 + RB, TBLK)):
                        tp = ppool.tile([P, P], BF16, tag="tp", name="tp", space="PSUM", bufs=1)
                        nc.tensor.transpose(
                            out=tp[:], in_=src_hT[:, blk * P:(blk + 1) * P],
                            identity=ident[:])
                        nc.scalar.copy(out=rows[:, blk - b0, :], in_=tp[:])
                    nb = min(b0 + RB, TBLK) - b0
                    nc.sync.dma_start(
                        out=ccv[:, b0:b0 + nb, :], in_=rows[:, :nb, :])
                nc.gpsimd.collective_compute(
                    "AllGather", mybir.AluOpType.bypass,
                    ins=[cc_in_t[:]], outs=[cc_out_t[:]], replica_groups=rg)

            def emit_layer(li, h_in, table, rootw, relw, bias_col, finish):
                # per-stream gather state
                cur = {}
                stcur = [-1, None]   # st-stream chunk state

                def ensure_chunk(r, b, blkloc):
                    ch = blkloc // NBLK_CH
                    key = (r, b)
                    if cur.get(key, (-1,))[0] == ch:
                        return cur[key]
                    gblk0 = int(pl.stream_blk_base[r, b]) + ch * NBLK_CH
                    it = wpool.tile([P, CHS // 16], I16, tag=f"idx{r}{b}", name=f"idx{r}{b}", bufs=3)
                    nc.sync.dma_start(
                        out=it[:],
                        in_=idxt[:, gblk0 * P // 16:(gblk0 + NBLK_CH) * P // 16])
                    gt = wpool.tile([P, NBLK_CH, P], BF16, tag=f"st{r}{b}", name=f"st{r}{b}", bufs=3)
                    # final stream chunk: trailing -1 idxs are dropped by the
                    # Q7 kernel; pass the matching valid count
                    realblk = int(pl.stream_nblk[r, b] - pl.padblk[r, b])
                    nvalid = min(CHS, (realblk - ch * NBLK_CH) * P)
                    nc.gpsimd.dma_gather(
                        out_ap=gt[:],
                        in_ap=table[b * BR:min((b + 1) * BR, TROWS), :],
                        idxs_ap=it[:], num_idxs=CHS, num_idxs_reg=nvalid,
                        elem_size=H, single_packet=False,
                        queue_num=(r * BANKS + b) % 4)
                    cur[key] = (ch, gt)
                    return cur[key]

                def ensure_st(m):
                    ch = m // ST_CH
                    if stcur[0] != ch:
                        sc = wpool.tile([P, ST_CH, WIN], BF16, tag="stc", name="stc", bufs=4)
                        nc.sync.dma_start(
                            out=sc[:], in_=stt[:, ch * ST_CH:(ch + 1) * ST_CH, :])
                        stcur[0] = ch
                        stcur[1] = sc
                    return stcur[1]

                m = 0
                for w in range(NW):
                    ws = slice(w * WIN, (w + 1) * WIN)
                    agg = []
                    for r in range(R):
                        pa = ppool.tile([P, WIN], F32, tag=f"agg{r}", name=f"agg{r}",
                                        space="PSUM")
                        nblk_w = int(pl.bpg[r, :, w].sum())
                        j = 0
                        for b in range(BANKS):
                            base = int(pl.group_blk_base[r, b, w]
                                       - pl.stream_blk_base[r, b])
                            for k in range(int(pl.bpg[r, b, w])):
                                blkloc = base + k
                                ch, gt = ensure_chunk(r, b, blkloc)
                                pos = blkloc - ch * NBLK_CH
                                sc = ensure_st(m)
                                nc.tensor.matmul(
                                    out=pa[:], lhsT=gt[:, pos, :],
                                    rhs=sc[:, m % ST_CH, :],
                                    start=(j == 0), stop=(j == nblk_w - 1))
                                m += 1
                                j += 1
                        asb = wpool.tile([P, WIN], BF16, tag=f"asb{r}", name=f"asb{r}", bufs=2)
                        if nblk_w == 0:
                            nc.vector.memset(asb[:], 0.0)
                        else:
                            nc.scalar.copy(out=asb[:], in_=pa[:])
                        agg.append(asb)

                    po = ppool.tile([P, WIN], F32, tag="po", name="po", space="PSUM")
                    nc.tensor.matmul(out=po[:], lhsT=rootw[:, 0, :],
                                     rhs=h_in[:, ws], start=True, stop=False)
                    for r in range(R):
                        nc.tensor.matmul(out=po[:], lhsT=relw[r][:, 0, :],
                                         rhs=agg[r][:], start=False,
                                         stop=(r == R - 1))
                    finish(w, ws, po)
                assert m == pl.EMIT_TOT

            def finish_h(h_out, bias_col):
                def fn(w, ws, po):
                    nc.scalar.activation(
                        out=h_out[:, ws], in_=po[:],
                        func=mybir.ActivationFunctionType.Identity,
                        bias=bias_t[:, bias_col:bias_col + 1])
                return fn

            def finish_cls(bias_col):
                # layer-2 output -> classifier -> DRAM, fused per window
                def fn(w, ws, po):
                    h2 = wpool.tile([P, WIN], BF16, tag="h2", name="h2", bufs=2)
                    nc.scalar.activation(
                        out=h2[:], in_=po[:],
                        func=mybir.ActivationFunctionType.Identity,
                        bias=bias_t[:, bias_col:bias_col + 1])
                    pc = ppool.tile([P, WIN], F32, tag="pc", name="pc", space="PSUM", bufs=1)
                    nc.tensor.matmul(out=pc[:], lhsT=wt["wcls"][:, 0, :],
                                     rhs=h2[:], start=True, stop=True)
                    oc = wpool.tile([P, WIN], F32, tag="oc", name="oc", bufs=2)
                    nc.scalar.activation(
                        out=oc[:], in_=pc[:],
                        func=mybir.ActivationFunctionType.Identity,
                        bias=bias_t[:, 8:9])
                    nc.sync.dma_start(out=outT[:, ws], in_=oc[:])
                return fn

            # table of h0 + layer 1
            emit_table(hT[0], cc_in[0], cc_out[0])
            emit_layer(0, hT[0], cc_out[0],
                       wt["root1"], [wt["rel10"], wt["rel11"]], 6,
                       finish_h(hT[1], 6))
            # table of h1 + layer 2 (classifier fused)
            emit_table(hT[1], cc_in[1], cc_out[1])
            emit_layer(1, hT[1], cc_out[1],
                       wt["root2"], [wt["rel20"], wt["rel21"]], 7,
                       finish_cls(7))

    nc.compile()
    return nc


# ---------------------------------------------------------------------------
# entry point
# ---------------------------------------------------------------------------

def kernel(**inputs):
    cfg = _derived(CFG)
    return _kernel_impl(inputs, cfg)


def _kernel_impl(inputs, cfg, trace=False):
    d = cfg
    NC, SH, SHP = d["NC"], d["SH"], d["SHP"]

    pl = build_plan(inputs["edge_index"], inputs["edge_type"], d)
    xs = prep_x(np.asarray(inputs["x"], np.float32), d)
    w = prep_weights(inputs, d)

    nc = build_bass(d, pl)

    in_maps = []
    for c in range(NC):
        m = {"xT": xs[c], "idxt": pl.idx16[c], "stt": pl.stT[c],
             "biases": w["biases"]}
        for nm in ["wdes", "wtweet", "wnum", "wcat", "win", "root1", "rel10",
                   "rel11", "root2", "rel20", "rel21", "wcls"]:
            m[nm] = w[nm]
        in_maps.append(m)

    res = run_bass_kernel_spmd(nc, in_maps, core_ids=list(range(NC)),
                               trace=trace)

    out = np.empty((NC * SH, d["H"]), np.float32)
    for c in range(NC):
        out[c * SH:(c + 1) * SH] = res.results[c]["outT"].T[:SH]
    if trace:
        return out, res
    return out


# revision 16
# speedup vs baseline: 1.0553x; 1.0553x over previous
"""BotRGCN Trainium2 kernel: feature transform + 2 RGCN layers + classifier.

Sharding: nodes split across 8 cores by id (12500/core, padded to 12544).
Edges partitioned by destination shard; per (relation, dst-window, src-bank)
groups padded to a block structure uniform across cores so a single SPMD
program serves all 8 cores. Source features exchanged via bf16 AllGather of
the per-layer node-feature table; gathers via int16 dma_gather per src bank,
round-robined over 4 SWDGE queues so descriptor generation uses all Q7 core
pairs. The per-block scatter one-hot (edge -> dst column, mean weight folded
in) is precomputed on host and streamed from DRAM, keeping DVE off the
critical path.
"""

import sys

sys.path.insert(0, "/opt/trn_rl_repo")

from contextlib import ExitStack

import numpy as np
import ml_dtypes

import concourse.bass as bass
import concourse.bacc as bacc
import concourse.mybir as mybir
import concourse.tile as tile
from concourse.masks import make_identity
from concourse.bass_utils import run_bass_kernel_spmd

BF16 = mybir.dt.bfloat16
F32 = mybir.dt.float32
I16 = mybir.dt.int16

P = 128

# full-problem config (test.py overrides for mini runs)
CFG = dict(
    N=100000,        # nodes
    NC=8,            # cores
    R=2,             # relations
    H=128,
    DES=768, TWEET=768, NUMP=6, CATP=11,
    WIN=256,         # dst window (PSUM free dim)
    NBLK_CH=12,      # gather-chunk size in 128-edge blocks
    ST_CH=8,         # st-stream chunk size in blocks
    BANKROWS=25088,  # gather-table bank rows (< 2^15)
    NTF=512,         # feature-stage node tile
)


def _derived(cfg):
    d = dict(cfg)
    d["SH"] = cfg["N"] // cfg["NC"]
    d["SHP"] = ((d["SH"] + P - 1) // P) * P
    d["NW"] = d["SHP"] // cfg["WIN"]
    assert d["SHP"] % cfg["WIN"] == 0
    d["TROWS"] = cfg["NC"] * d["SHP"]           # padded table rows
    d["BANKS"] = (d["TROWS"] + cfg["BANKROWS"] - 1) // cfg["BANKROWS"]
    d["TBLK"] = d["SHP"] // P                   # 128-row blobs per core
    # x feature layout: [des | tweet | num(pad to 128) | cat(pad to 128)]
    d["KDES"] = cfg["DES"] // P
    d["KTWEET"] = cfg["TWEET"] // P
    d["KX"] = d["KDES"] + d["KTWEET"] + 2
    d["XROWS"] = d["KX"] * P
    return d


# ---------------------------------------------------------------------------
# host-side graph planning
# ---------------------------------------------------------------------------

class Plan:
    pass


def build_plan(edge_index, edge_type, cfg):
    """Group edges per core by (rel, dst-window, src-bank); pad each group to a
    whole number of 128-edge blocks, uniform across cores. Returns per-core
    gather-index arrays, the streamed scatter one-hot tiles (emission order),
    plus the uniform block structure."""
    d = cfg
    NC, SH, SHP, WIN, NW = d["NC"], d["SH"], d["SHP"], d["WIN"], d["NW"]
    BANKS, BR, NBLK_CH = d["BANKS"], d["BANKROWS"], d["NBLK_CH"]
    ST_CH = d["ST_CH"]
    R = d["R"]
    N = d["N"]
    TBLK = d["TBLK"]

    src = np.asarray(edge_index[0], dtype=np.int64)
    dst = np.asarray(edge_index[1], dtype=np.int64)
    et = np.asarray(edge_type, dtype=np.int64)

    core = dst // SH
    dl = dst - core * SH
    # table row of a (padded) node: blob layout [p][t] per core
    sl = src - (src // SH) * SH
    ps = (src // SH) * SHP + (sl % P) * TBLK + (sl // P)
    bank = ps // BR
    bidx = (ps - bank * BR).astype(np.int16)
    win = dl // WIN
    dw = (dl - win * WIN).astype(np.int64)

    # per-(rel, node) in-degree -> per-edge mean weight
    cnt = np.bincount(et * N + dst, minlength=R * N).reshape(R, N)
    wv = (1.0 / np.maximum(cnt, 1.0))[et, dst].astype(np.float32)

    # group = (rel, bank, win); uniform EXACT sizes = max count over cores.
    # Groups are laid out back-to-back within each (rel, bank) stream (no
    # 128-alignment): a 128-slot block may span several groups; each
    # (group, block) pair becomes one matmul emission whose streamed st tile
    # is zero outside the group's slots.
    NG = R * BANKS * NW
    gid = (et * BANKS + bank) * NW + win
    counts = np.bincount(core * NG + gid, minlength=NC * NG).reshape(NC, NG)
    gsz = counts.max(axis=0).reshape(R, BANKS, NW).astype(np.int64)

    CHSL = NBLK_CH * P             # slots per gather chunk
    group_slot_base = np.zeros((R, BANKS, NW), np.int64)
    stream_blk_base = np.zeros((R, BANKS), np.int64)   # block units
    stream_nblk = np.zeros((R, BANKS), np.int64)       # incl. pad, blocks
    stream_vblk = np.zeros((R, BANKS), np.int64)       # gathered blocks
    base = 0                                           # slot units
    for r in range(R):
        for b in range(BANKS):
            stream_blk_base[r, b] = base // P
            v = 0
            for w in range(NW):
                group_slot_base[r, b, w] = base + v
                v += int(gsz[r, b, w])
            vpad = ((v + CHSL - 1) // CHSL) * CHSL
            if vpad == 0:
                vpad = CHSL
            stream_vblk[r, b] = (v + P - 1) // P
            stream_nblk[r, b] = vpad // P
            base += vpad
    TOTSLOT = base
    TOTBLK = TOTSLOT // P

    # emission list: m over (w, r, b, block-of-group)
    emis_rw = {}
    m_base = np.zeros((R, BANKS, NW), np.int64)
    first_blk = np.zeros((R, BANKS, NW), np.int64)
    m = 0
    for w in range(NW):
        for r in range(R):
            lst = []
            for b in range(BANKS):
                a = int(group_slot_base[r, b, w])
                e = a + int(gsz[r, b, w])
                if e > a:
                    B0, B1 = a // P, (e - 1) // P
                    m_base[r, b, w] = m
                    first_blk[r, b, w] = B0
                    for B in range(B0, B1 + 1):
                        lst.append((b, B))
                        m += 1
            emis_rw[(r, w)] = lst
    EMIT_TOT = m
    EMIT_PAD = ((EMIT_TOT + ST_CH - 1) // ST_CH) * ST_CH

    okey = core * NG + gid
    order = np.argsort(okey, kind="stable")
    so = okey[order]
    first_of = np.r_[True, so[1:] != so[:-1]]
    idx_in_run = np.arange(len(so)) - np.maximum.accumulate(
        np.where(first_of, np.arange(len(so)), 0)
    )
    gsb_flat = group_slot_base.transpose(0, 1, 2).reshape(-1)  # [r,b,w] order
    # gid is (r*BANKS+b)*NW+w which matches the flat [r,b,w] layout
    slot = gsb_flat[so % NG] + idx_in_run

    idx16 = np.zeros((NC, 8 * 16, TOTSLOT // 16), np.int16)
    # slots beyond each stream's gathered region: -1 (trailing, trimmed)
    for r in range(R):
        for b in range(BANKS):
            p0 = (stream_blk_base[r, b] + stream_vblk[r, b]) * P
            p1 = (stream_blk_base[r, b] + stream_nblk[r, b]) * P
            if p1 > p0:
                s = np.arange(p0, p1)
                for g in range(8):
                    idx16[:, 16 * g + (s % 16), s // 16] = -1

    ecore = core[order]
    col = slot // 16
    prow = (slot % 16).astype(np.int64)
    for g in range(8):
        idx16[ecore, 16 * g + prow, col] = bidx[order]

    # streamed scatter tiles, emission order: stT[p, m, j]
    mb_flat = m_base.reshape(-1)
    fb_flat = first_blk.reshape(-1)
    m_edge = mb_flat[so % NG] + (slot // P - fb_flat[so % NG])
    stT = np.zeros((NC, P, EMIT_PAD, WIN), ml_dtypes.bfloat16)
    stT[ecore, slot % P, m_edge, dw[order]] = wv[order]

    pl = Plan()
    pl.idx16 = idx16.reshape(NC, P, TOTSLOT // 16)
    pl.stT = stT
    pl.emis_rw = emis_rw
    pl.TOTBLK = TOTBLK
    pl.EMIT_TOT = EMIT_TOT
    pl.EMIT_PAD = EMIT_PAD
    pl.stream_blk_base = stream_blk_base
    pl.stream_nblk = stream_nblk
    pl.stream_vblk = stream_vblk
    return pl


def prep_x(x, cfg):
    """Per-core transposed bf16 feature blocks [XROWS, SHP]."""
    d = cfg
    NC, SH, SHP = d["NC"], d["SH"], d["SHP"]
    NUMP, TWEET, CATP, DES = d["NUMP"], d["TWEET"], d["CATP"], d["DES"]
    KD, KT = d["KDES"], d["KTWEET"]
    out = np.zeros((NC, d["XROWS"], SHP), ml_dtypes.bfloat16)
    for c in range(NC):
        xs = x[c * SH:(c + 1) * SH]
        xT = np.zeros((d["XROWS"], SHP), np.float32)
        xT[:DES, :SH] = xs[:, NUMP + TWEET + CATP:].T
        xT[DES:DES + TWEET, :SH] = xs[:, NUMP:NUMP + TWEET].T
        xT[(KD + KT) * P:(KD + KT) * P + NUMP, :SH] = xs[:, :NUMP].T
        xT[(KD + KT + 1) * P:(KD + KT + 1) * P + CATP, :SH] = \
            xs[:, NUMP + TWEET:NUMP + TWEET + CATP].T
        out[c] = xT.astype(ml_dtypes.bfloat16)
    return out


def prep_weights(inp, cfg):
    """bf16 weight blocks + packed fp32 biases."""
    bf = lambda a: np.asarray(a, np.float32).astype(ml_dtypes.bfloat16)
    d = cfg
    wnum = np.zeros((P, d["H"]), np.float32)
    wnum[:d["NUMP"]] = inp["W_num"]
    wcat = np.zeros((P, d["H"]), np.float32)
    wcat[:d["CATP"]] = inp["W_cat"]
    w = {
        "wdes": bf(inp["W_des"]), "wtweet": bf(inp["W_tweet"]),
        "wnum": bf(wnum), "wcat": bf(wcat), "win": bf(inp["W_in"]),
        "root1": bf(inp["root1"]), "rel10": bf(inp["rel1"][0]),
        "rel11": bf(inp["rel1"][1]),
        "root2": bf(inp["root2"]), "rel20": bf(inp["rel2"][0]),
        "rel21": bf(inp["rel2"][1]), "wcls": bf(inp["W_cls"]),
    }
    biases = np.stack(
        [inp["b_des"], inp["b_tweet"], inp["b_num"], inp["b_cat"],
         inp["b_in"], inp["prelu_a"], inp["bias1"], inp["bias2"],
         inp["b_cls"]], axis=1).astype(np.float32)   # [128, 9]
    w["biases"] = biases
    return w


# ---------------------------------------------------------------------------
# bass program
# ---------------------------------------------------------------------------

def build_bass(cfg, pl):
    d = cfg
    NC, SHP, WIN, NW, NTF = d["NC"], d["SHP"], d["WIN"], d["NW"], d["NTF"]
    BANKS, BR, NBLK_CH = d["BANKS"], d["BANKROWS"], d["NBLK_CH"]
    ST_CH = d["ST_CH"]
    R, H = d["R"], d["H"]
    KD, KT, KX = d["KDES"], d["KTWEET"], d["KX"]
    TBLK = d["TBLK"]
    TROWS = d["TROWS"]
    CHS = NBLK_CH * P      # idx slots per chunk

    nc = bacc.Bacc(None, target_bir_lowering=False, debug=False,
                   num_devices=NC, num_swdge_queues=4,
                   dynamic_dma_scratch_size=65536)

    # ---- I/O ----
    xT = nc.dram_tensor("xT", [d["XROWS"], SHP], BF16, kind="ExternalInput")
    idxt = nc.dram_tensor("idxt", [P, pl.TOTBLK * P // 16], I16, kind="ExternalInput")
    stt = nc.dram_tensor("stt", [P, pl.EMIT_PAD, WIN], BF16, kind="ExternalInput")
    wts = {}
    for nm, shp in [("wdes", [d["DES"], H]), ("wtweet", [d["TWEET"], H]),
                    ("wnum", [P, H]), ("wcat", [P, H]), ("win", [4 * P, H]),
                    ("root1", [H, H]), ("rel10", [H, H]), ("rel11", [H, H]),
                    ("root2", [H, H]), ("rel20", [H, H]), ("rel21", [H, H]),
                    ("wcls", [H, H])]:
        wts[nm] = nc.dram_tensor(nm, shp, BF16, kind="ExternalInput")
    biases = nc.dram_tensor("biases", [P, 9], F32, kind="ExternalInput")
    outT = nc.dram_tensor("outT", [P, SHP], F32, kind="ExternalOutput")

    # ---- collective tables ----
    cc_in = [nc.dram_tensor(f"cc{i}_in", [SHP, H], BF16, kind="Internal")
             for i in (1, 2)]
    cc_out = [nc.dram_tensor(f"cc{i}_out", [NC * SHP, H], BF16,
                             kind="Internal", addr_space="Shared")
              for i in (1, 2)]

    rg = [list(range(NC))]

    with tile.TileContext(nc) as tc:
        with (
            tc.tile_pool(name="const", bufs=1) as cpool,
            tc.tile_pool(name="resident", bufs=1) as rpool,
            ExitStack() as mstack,
        ):
            # ---- constants ----
            ident = cpool.tile([P, P], BF16)
            make_identity(nc, ident[:])
            bias_t = cpool.tile([P, 9], F32)
            nc.sync.dma_start(out=bias_t[:], in_=biases[:])

            wt = {}
            for nm, kb in [("wdes", KD), ("wtweet", KT), ("wnum", 1),
                           ("wcat", 1), ("win", 4), ("root1", 1),
                           ("rel10", 1), ("rel11", 1), ("root2", 1),
                           ("rel20", 1), ("rel21", 1), ("wcls", 1)]:
                t = cpool.tile([P, kb, H], BF16, tag=f"w_{nm}", name=f"w_{nm}")
                nc.sync.dma_start(
                    out=t[:], in_=wts[nm].rearrange("(k p) h -> p k h", p=P))
                wt[nm] = t

            # resident activations (transposed, [H, SHP] bf16)
            hT = [rpool.tile([P, SHP], BF16, tag="ht", name=f"hT{i}", bufs=2)
                  for i in range(2)]

            # =============== feature transform ===============
            fstack = ExitStack()
            fpool = fstack.enter_context(tc.tile_pool(name="featsb", bufs=2))
            fpp = fstack.enter_context(
                tc.tile_pool(name="featps", bufs=2, space="PSUM"))
            ntiles = (SHP + NTF - 1) // NTF
            for t in range(ntiles):
                n0 = t * NTF
                n1 = min(SHP, n0 + NTF)
                nn = n1 - n0
                xt = fpool.tile([P, KX, NTF], BF16, tag="xt", name="xt")
                nc.sync.dma_start(
                    out=xt[:, :, :nn],
                    in_=xT.rearrange("(k p) n -> p k n", p=P)[:, :, n0:n1])

                zb = []
                for bi, (wnm, ks, kn) in enumerate([
                        ("wdes", 0, KD), ("wtweet", KD, KT),
                        ("wnum", KD + KT, 1), ("wcat", KD + KT + 1, 1)]):
                    pz = fpp.tile([P, NTF], F32, tag=f"pz{bi}", name=f"pz{bi}", space="PSUM", bufs=1)
                    for k in range(kn):
                        nc.tensor.matmul(
                            out=pz[:, :nn], lhsT=wt[wnm][:, k, :],
                            rhs=xt[:, ks + k, :nn],
                            start=(k == 0), stop=(k == kn - 1))
                    v = fpool.tile([P, NTF], BF16, tag=f"v{bi}", name=f"v{bi}")
                    nc.scalar.activation(
                        out=v[:, :nn], in_=pz[:, :nn],
                        func=mybir.ActivationFunctionType.Identity,
                        bias=bias_t[:, bi:bi + 1])
                    z = fpool.tile([P, NTF], BF16, tag=f"z{bi}", name=f"z{bi}")
                    nc.vector.scalar_tensor_tensor(
                        out=z[:, :nn], in0=v[:, :nn], scalar=0.01,
                        in1=v[:, :nn], op0=mybir.AluOpType.mult,
                        op1=mybir.AluOpType.max)
                    zb.append(z)

                ph = fpp.tile([P, NTF], F32, tag="ph", name="ph", space="PSUM")
                for k in range(4):
                    nc.tensor.matmul(out=ph[:, :nn], lhsT=wt["win"][:, k, :],
                                     rhs=zb[k][:, :nn],
                                     start=(k == 0), stop=(k == 3))
                vh = fpool.tile([P, NTF], F32, tag="vh", name="vh")
                nc.scalar.activation(
                    out=vh[:, :nn], in_=ph[:, :nn],
                    func=mybir.ActivationFunctionType.Identity,
                    bias=bias_t[:, 4:5])
                nc.vector.scalar_tensor_tensor(
                    out=hT[0][:, n0:n1], in0=vh[:, :nn],
                    scalar=bias_t[:, 5:6], in1=vh[:, :nn],
                    op0=mybir.AluOpType.mult, op1=mybir.AluOpType.max)

            fstack.close()
            wpool = mstack.enter_context(tc.tile_pool(name="work", bufs=3))
            ppool = mstack.enter_context(
                tc.tile_pool(name="psum", bufs=2, space="PSUM"))

            # =============== per-layer helper ===============
            def emit_table(src_hT, cc_in_t, cc_out_t):
                RB = 7   # transpose blocks per staging tile / DMA
                ccv = cc_in_t.rearrange("(p t) h -> p t h", p=P)
                for b0 in range(0, TBLK, RB):
                    rows = wpool.tile([P, RB, P], BF16, tag="rows",
                                      name="rows", bufs=2)
                    for blk in range(b0, min(b0 + RB, TBLK)):
                        tp = ppool.tile([P, P], BF16, tag="tp", name="tp", space="PSUM", bufs=1)
                        nc.tensor.transpose(
                            out=tp[:], in_=src_hT[:, blk * P:(blk + 1) * P],
                            identity=ident[:])
                        nc.scalar.copy(out=rows[:, blk - b0, :], in_=tp[:])
                    nb = min(b0 + RB, TBLK) - b0
                    nc.sync.dma_start(
                        out=ccv[:, b0:b0 + nb, :], in_=rows[:, :nb, :])
                nc.gpsimd.collective_compute(
                    "AllGather", mybir.AluOpType.bypass,
                    ins=[cc_in_t[:]], outs=[cc_out_t[:]], replica_groups=rg)

            def emit_layer(li, h_in, table, rootw, relw, bias_col, finish):
                # per-stream gather state
                cur = {}
                stcur = [-1, None]   # st-stream chunk state

                def ensure_chunk(r, b, blkloc):
                    ch = blkloc // NBLK_CH
                    key = (r, b)
                    if cur.get(key, (-1,))[0] == ch:
                        return cur[key]
                    gblk0 = int(pl.stream_blk_base[r, b]) + ch * NBLK_CH
                    it = wpool.tile([P, CHS // 16], I16, tag=f"idx{r}{b}", name=f"idx{r}{b}", bufs=2)
                    nc.sync.dma_start(
                        out=it[:],
                        in_=idxt[:, gblk0 * P // 16:(gblk0 + NBLK_CH) * P // 16])
                    gt = wpool.tile([P, NBLK_CH, P], BF16, tag=f"st{r}{b}", name=f"st{r}{b}", bufs=2)
                    # final stream chunk: trailing -1 idxs are dropped by the
                    # Q7 kernel; pass the matching valid count
                    nvalid = min(CHS, int(pl.stream_vblk[r, b]
                                          - ch * NBLK_CH) * P)
                    nc.gpsimd.dma_gather(
                        out_ap=gt[:],
                        in_ap=table[b * BR:min((b + 1) * BR, TROWS), :],
                        idxs_ap=it[:], num_idxs=CHS, num_idxs_reg=nvalid,
                        elem_size=H, single_packet=False,
                        queue_num=(r * BANKS + b) % 4)
                    cur[key] = (ch, gt)
                    return cur[key]

                def ensure_st(m):
                    ch = m // ST_CH
                    if stcur[0] != ch:
                        sc = wpool.tile([P, ST_CH, WIN], BF16, tag="stc", name="stc", bufs=10)
                        nc.scalar.dma_start(
                            out=sc[:], in_=stt[:, ch * ST_CH:(ch + 1) * ST_CH, :])
                        stcur[0] = ch
                        stcur[1] = sc
                    return stcur[1]

                def emit_tail(w, agg):
                    # root+rel matmuls and output chain for window w; emitted
                    # one window late so PE never head-of-line blocks on the
                    # Scalar psum->sbuf copies.
                    ws = slice(w * WIN, (w + 1) * WIN)
                    po = ppool.tile([P, WIN], F32, tag="po", name="po", space="PSUM")
                    nc.tensor.matmul(out=po[:], lhsT=rootw[:, 0, :],
                                     rhs=h_in[:, ws], start=True, stop=False)
                    for r in range(R):
                        nc.tensor.matmul(out=po[:], lhsT=relw[r][:, 0, :],
                                         rhs=agg[r][:], start=False,
                                         stop=(r == R - 1))
                    finish(w, ws, po)

                m = 0
                prev = None
                for w in range(NW):
                    agg = []
                    for r in range(R):
                        pa = ppool.tile([P, WIN], F32, tag=f"agg{r}", name=f"agg{r}",
                                        space="PSUM")
                        lst = pl.emis_rw[(r, w)]
                        nblk_w = len(lst)
                        for j, (b, Bg) in enumerate(lst):
                            blkloc = Bg - int(pl.stream_blk_base[r, b])
                            ch, gt = ensure_chunk(r, b, blkloc)
                            pos = blkloc - ch * NBLK_CH
                            sc = ensure_st(m)
                            nc.tensor.matmul(
                                out=pa[:], lhsT=gt[:, pos, :],
                                rhs=sc[:, m % ST_CH, :],
                                start=(j == 0), stop=(j == nblk_w - 1))
                            m += 1
                        asb = wpool.tile([P, WIN], BF16, tag=f"asb{r}", name=f"asb{r}", bufs=2)
                        if nblk_w == 0:
                            nc.vector.memset(asb[:], 0.0)
                        else:
                            nc.scalar.copy(out=asb[:], in_=pa[:])
                        agg.append(asb)

                    if prev is not None:
                        emit_tail(prev[0], prev[1])
                    prev = (w, agg)
                emit_tail(prev[0], prev[1])
                assert m == pl.EMIT_TOT

            def finish_h(h_out, bias_col):
                def fn(w, ws, po):
                    nc.scalar.activation(
                        out=h_out[:, ws], in_=po[:],
                        func=mybir.ActivationFunctionType.Identity,
                        bias=bias_t[:, bias_col:bias_col + 1])
                return fn

            def finish_cls(bias_col):
                # layer-2 output -> classifier -> DRAM, fused per window
                def fn(w, ws, po):
                    h2 = wpool.tile([P, WIN], BF16, tag="h2", name="h2", bufs=2)
                    nc.scalar.activation(
                        out=h2[:], in_=po[:],
                        func=mybir.ActivationFunctionType.Identity,
                        bias=bias_t[:, bias_col:bias_col + 1])
                    pc = ppool.tile([P, WIN], F32, tag="pc", name="pc", space="PSUM", bufs=1)
                    nc.tensor.matmul(out=pc[:], lhsT=wt["wcls"][:, 0, :],
                                     rhs=h2[:], start=True, stop=True)
                    oc = wpool.tile([P, WIN], F32, tag="oc", name="oc", bufs=2)
                    nc.scalar.activation(
                        out=oc[:], in_=pc[:],
                        func=mybir.ActivationFunctionType.Identity,
                        bias=bias_t[:, 8:9])
                    nc.sync.dma_start(out=outT[:, ws], in_=oc[:])
                return fn

            # table of h0 + layer 1
            emit_table(hT[0], cc_in[0], cc_out[0])
            emit_layer(0, hT[0], cc_out[0],
                       wt["root1"], [wt["rel10"], wt["rel11"]], 6,
                       finish_h(hT[1], 6))
            # table of h1 + layer 2 (classifier fused)
            emit_table(hT[1], cc_in[1], cc_out[1])
            emit_layer(1, hT[1], cc_out[1],
                       wt["root2"], [wt["rel20"], wt["rel21"]], 7,
                       finish_cls(7))

    nc.compile()
    return nc


# ---------------------------------------------------------------------------
# entry point
# ---------------------------------------------------------------------------

def kernel(**inputs):
    cfg = _derived(CFG)
    return _kernel_impl(inputs, cfg)


def _kernel_impl(inputs, cfg, trace=False):
    d = cfg
    NC, SH, SHP = d["NC"], d["SH"], d["SHP"]

    pl = build_plan(inputs["edge_index"], inputs["edge_type"], d)
    xs = prep_x(np.asarray(inputs["x"], np.float32), d)
    w = prep_weights(inputs, d)

    nc = build_bass(d, pl)

    in_maps = []
    for c in range(NC):
        m = {"xT": xs[c], "idxt": pl.idx16[c], "stt": pl.stT[c],
             "biases": w["biases"]}
        for nm in ["wdes", "wtweet", "wnum", "wcat", "win", "root1", "rel10",
                   "rel11", "root2", "rel20", "rel21", "wcls"]:
            m[nm] = w[nm]
        in_maps.append(m)

    res = run_bass_kernel_spmd(nc, in_maps, core_ids=list(range(NC)),
                               trace=trace)

    out = np.empty((NC * SH, d["H"]), np.float32)
    for c in range(NC):
        out[c * SH:(c + 1) * SH] = res.results[c]["outT"].T[:SH]
    if trace:
        return out, res
    return out


# revision 17
# speedup vs baseline: 1.2341x; 1.1695x over previous
"""BotRGCN Trainium2 kernel: feature transform + 2 RGCN layers + classifier.

Sharding: nodes split across 8 cores by id (12500/core, padded to 12544).
Edges partitioned by destination shard; per (relation, dst-window, src-bank)
groups padded to a block structure uniform across cores so a single SPMD
program serves all 8 cores. Source features exchanged via bf16 AllGather of
the per-layer node-feature table; gathers via int16 dma_gather per src bank,
round-robined over 4 SWDGE queues so descriptor generation uses all Q7 core
pairs. The per-block scatter one-hot (edge -> dst column, mean weight folded
in) is precomputed on host and streamed from DRAM, keeping DVE off the
critical path.
"""

import sys

sys.path.insert(0, "/opt/trn_rl_repo")

from contextlib import ExitStack

import numpy as np
import ml_dtypes

import concourse.bass as bass
import concourse.bacc as bacc
import concourse.mybir as mybir
import concourse.tile as tile
from concourse.masks import make_identity
from concourse.bass_utils import run_bass_kernel_spmd

BF16 = mybir.dt.bfloat16
F32 = mybir.dt.float32
I16 = mybir.dt.int16

P = 128

# full-problem config (test.py overrides for mini runs)
CFG = dict(
    N=100000,        # nodes
    NC=8,            # cores
    R=2,             # relations
    H=128,
    DES=768, TWEET=768, NUMP=6, CATP=11,
    WIN=256,         # dst window (PSUM free dim)
    NBLK_CH=12,      # gather-chunk size in 128-edge blocks
    ST_CH=8,         # st-stream chunk size in blocks
    BANKROWS=25088,  # gather-table bank rows (< 2^15)
    NTF=512,         # feature-stage node tile
)


def _derived(cfg):
    d = dict(cfg)
    d["SH"] = cfg["N"] // cfg["NC"]
    d["SHP"] = ((d["SH"] + P - 1) // P) * P
    d["NW"] = d["SHP"] // cfg["WIN"]
    assert d["SHP"] % cfg["WIN"] == 0
    d["TROWS"] = cfg["NC"] * d["SHP"]           # padded table rows
    d["BANKS"] = (d["TROWS"] + cfg["BANKROWS"] - 1) // cfg["BANKROWS"]
    d["TBLK"] = d["SHP"] // P                   # 128-row blobs per core
    # x feature layout: [des | tweet | num(pad to 128) | cat(pad to 128)]
    d["KDES"] = cfg["DES"] // P
    d["KTWEET"] = cfg["TWEET"] // P
    d["KX"] = d["KDES"] + d["KTWEET"] + 2
    d["XROWS"] = d["KX"] * P
    return d


# ---------------------------------------------------------------------------
# host-side graph planning
# ---------------------------------------------------------------------------

class Plan:
    pass


def build_plan(edge_index, edge_type, cfg):
    """Group edges per core by (rel, dst-window, src-bank); pad each group to a
    whole number of 128-edge blocks, uniform across cores. Returns per-core
    gather-index arrays, the streamed scatter one-hot tiles (emission order),
    plus the uniform block structure."""
    d = cfg
    NC, SH, SHP, WIN, NW = d["NC"], d["SH"], d["SHP"], d["WIN"], d["NW"]
    BANKS, BR, NBLK_CH = d["BANKS"], d["BANKROWS"], d["NBLK_CH"]
    ST_CH = d["ST_CH"]
    R = d["R"]
    N = d["N"]
    TBLK = d["TBLK"]

    src = np.asarray(edge_index[0], dtype=np.int64)
    dst = np.asarray(edge_index[1], dtype=np.int64)
    et = np.asarray(edge_type, dtype=np.int64)

    core = dst // SH
    dl = dst - core * SH
    # table row of a (padded) node: blob layout [p][t] per core
    sl = src - (src // SH) * SH
    ps = (src // SH) * SHP + (sl % P) * TBLK + (sl // P)
    bank = ps // BR
    bidx = (ps - bank * BR).astype(np.int16)
    win = dl // WIN
    dw = (dl - win * WIN).astype(np.int64)

    # per-(rel, node) in-degree -> per-edge mean weight
    cnt = np.bincount(et * N + dst, minlength=R * N).reshape(R, N)
    wv = (1.0 / np.maximum(cnt, 1.0))[et, dst].astype(np.float32)

    # group = (rel, bank, win); uniform EXACT sizes = max count over cores.
    # Groups are laid out back-to-back within each (rel, bank) stream (no
    # 128-alignment): a 128-slot block may span several groups; each
    # (group, block) pair becomes one matmul emission whose streamed st tile
    # is zero outside the group's slots.
    NG = R * BANKS * NW
    gid = (et * BANKS + bank) * NW + win
    counts = np.bincount(core * NG + gid, minlength=NC * NG).reshape(NC, NG)
    gsz = counts.max(axis=0).reshape(R, BANKS, NW).astype(np.int64)

    CHSL = NBLK_CH * P             # slots per gather chunk
    group_slot_base = np.zeros((R, BANKS, NW), np.int64)
    stream_blk_base = np.zeros((R, BANKS), np.int64)   # block units
    stream_nblk = np.zeros((R, BANKS), np.int64)       # incl. pad, blocks
    stream_vblk = np.zeros((R, BANKS), np.int64)       # gathered blocks
    base = 0                                           # slot units
    for r in range(R):
        for b in range(BANKS):
            stream_blk_base[r, b] = base // P
            v = 0
            for w in range(NW):
                group_slot_base[r, b, w] = base + v
                v += int(gsz[r, b, w])
            vpad = ((v + CHSL - 1) // CHSL) * CHSL
            if vpad == 0:
                vpad = CHSL
            stream_vblk[r, b] = (v + P - 1) // P
            stream_nblk[r, b] = vpad // P
            base += vpad
    TOTSLOT = base
    TOTBLK = TOTSLOT // P

    # emission list: m over (w, r, b, block-of-group)
    emis_rw = {}
    m_base = np.zeros((R, BANKS, NW), np.int64)
    first_blk = np.zeros((R, BANKS, NW), np.int64)
    m = 0
    for w in range(NW):
        for r in range(R):
            lst = []
            for b in range(BANKS):
                a = int(group_slot_base[r, b, w])
                e = a + int(gsz[r, b, w])
                if e > a:
                    B0, B1 = a // P, (e - 1) // P
                    m_base[r, b, w] = m
                    first_blk[r, b, w] = B0
                    for B in range(B0, B1 + 1):
                        lst.append((b, B))
                        m += 1
            emis_rw[(r, w)] = lst
    EMIT_TOT = m
    EMIT_PAD = ((EMIT_TOT + ST_CH - 1) // ST_CH) * ST_CH

    okey = core * NG + gid
    order = np.argsort(okey, kind="stable")
    so = okey[order]
    first_of = np.r_[True, so[1:] != so[:-1]]
    idx_in_run = np.arange(len(so)) - np.maximum.accumulate(
        np.where(first_of, np.arange(len(so)), 0)
    )
    gsb_flat = group_slot_base.transpose(0, 1, 2).reshape(-1)  # [r,b,w] order
    # gid is (r*BANKS+b)*NW+w which matches the flat [r,b,w] layout
    slot = gsb_flat[so % NG] + idx_in_run

    idx16 = np.zeros((NC, 8 * 16, TOTSLOT // 16), np.int16)
    # slots beyond each stream's gathered region: -1 (trailing, trimmed)
    for r in range(R):
        for b in range(BANKS):
            p0 = (stream_blk_base[r, b] + stream_vblk[r, b]) * P
            p1 = (stream_blk_base[r, b] + stream_nblk[r, b]) * P
            if p1 > p0:
                s = np.arange(p0, p1)
                for g in range(8):
                    idx16[:, 16 * g + (s % 16), s // 16] = -1

    ecore = core[order]
    col = slot // 16
    prow = (slot % 16).astype(np.int64)
    for g in range(8):
        idx16[ecore, 16 * g + prow, col] = bidx[order]

    # streamed scatter tiles, emission order: stT[p, m, j]
    mb_flat = m_base.reshape(-1)
    fb_flat = first_blk.reshape(-1)
    m_edge = mb_flat[so % NG] + (slot // P - fb_flat[so % NG])
    stT = np.zeros((NC, P, EMIT_PAD, WIN), ml_dtypes.bfloat16)
    stT[ecore, slot % P, m_edge, dw[order]] = wv[order]

    pl = Plan()
    pl.idx16 = idx16.reshape(NC, P, TOTSLOT // 16)
    pl.stT = stT
    pl.emis_rw = emis_rw
    pl.TOTBLK = TOTBLK
    pl.EMIT_TOT = EMIT_TOT
    pl.EMIT_PAD = EMIT_PAD
    pl.stream_blk_base = stream_blk_base
    pl.stream_nblk = stream_nblk
    pl.stream_vblk = stream_vblk
    return pl


def prep_x(x, cfg):
    """Per-core transposed bf16 feature blocks [XROWS, SHP]."""
    d = cfg
    NC, SH, SHP = d["NC"], d["SH"], d["SHP"]
    NUMP, TWEET, CATP, DES = d["NUMP"], d["TWEET"], d["CATP"], d["DES"]
    KD, KT = d["KDES"], d["KTWEET"]
    out = np.zeros((NC, d["XROWS"], SHP), ml_dtypes.bfloat16)
    for c in range(NC):
        xs = x[c * SH:(c + 1) * SH]
        xT = np.zeros((d["XROWS"], SHP), np.float32)
        xT[:DES, :SH] = xs[:, NUMP + TWEET + CATP:].T
        xT[DES:DES + TWEET, :SH] = xs[:, NUMP:NUMP + TWEET].T
        xT[(KD + KT) * P:(KD + KT) * P + NUMP, :SH] = xs[:, :NUMP].T
        xT[(KD + KT + 1) * P:(KD + KT + 1) * P + CATP, :SH] = \
            xs[:, NUMP + TWEET:NUMP + TWEET + CATP].T
        out[c] = xT.astype(ml_dtypes.bfloat16)
    return out


def prep_weights(inp, cfg):
    """bf16 weight blocks + packed fp32 biases."""
    bf = lambda a: np.asarray(a, np.float32).astype(ml_dtypes.bfloat16)
    d = cfg
    wnum = np.zeros((P, d["H"]), np.float32)
    wnum[:d["NUMP"]] = inp["W_num"]
    wcat = np.zeros((P, d["H"]), np.float32)
    wcat[:d["CATP"]] = inp["W_cat"]
    w = {
        "wdes": bf(inp["W_des"]), "wtweet": bf(inp["W_tweet"]),
        "wnum": bf(wnum), "wcat": bf(wcat), "win": bf(inp["W_in"]),
        "root1": bf(inp["root1"]), "rel10": bf(inp["rel1"][0]),
        "rel11": bf(inp["rel1"][1]),
        "root2": bf(inp["root2"]), "rel20": bf(inp["rel2"][0]),
        "rel21": bf(inp["rel2"][1]), "wcls": bf(inp["W_cls"]),
    }
    biases = np.stack(
        [inp["b_des"], inp["b_tweet"], inp["b_num"], inp["b_cat"],
         inp["b_in"], inp["prelu_a"], inp["bias1"], inp["bias2"],
         inp["b_cls"]], axis=1).astype(np.float32)   # [128, 9]
    w["biases"] = biases
    return w


# ---------------------------------------------------------------------------
# bass program
# ---------------------------------------------------------------------------

def build_bass(cfg, pl):
    d = cfg
    NC, SHP, WIN, NW, NTF = d["NC"], d["SHP"], d["WIN"], d["NW"], d["NTF"]
    BANKS, BR, NBLK_CH = d["BANKS"], d["BANKROWS"], d["NBLK_CH"]
    ST_CH = d["ST_CH"]
    R, H = d["R"], d["H"]
    KD, KT, KX = d["KDES"], d["KTWEET"], d["KX"]
    TBLK = d["TBLK"]
    TROWS = d["TROWS"]
    CHS = NBLK_CH * P      # idx slots per chunk

    nc = bacc.Bacc(None, target_bir_lowering=False, debug=False,
                   num_devices=NC, num_swdge_queues=4,
                   dynamic_dma_scratch_size=32768)

    # ---- I/O ----
    xT = nc.dram_tensor("xT", [d["XROWS"], SHP], BF16, kind="ExternalInput")
    idxt = nc.dram_tensor("idxt", [P, pl.TOTBLK * P // 16], I16, kind="ExternalInput")
    stt = nc.dram_tensor("stt", [P, pl.EMIT_PAD, WIN], BF16, kind="ExternalInput")
    wts = {}
    for nm, shp in [("wdes", [d["DES"], H]), ("wtweet", [d["TWEET"], H]),
                    ("wnum", [P, H]), ("wcat", [P, H]), ("win", [4 * P, H]),
                    ("root1", [H, H]), ("rel10", [H, H]), ("rel11", [H, H]),
                    ("root2", [H, H]), ("rel20", [H, H]), ("rel21", [H, H]),
                    ("wcls", [H, H])]:
        wts[nm] = nc.dram_tensor(nm, shp, BF16, kind="ExternalInput")
    biases = nc.dram_tensor("biases", [P, 9], F32, kind="ExternalInput")
    outT = nc.dram_tensor("outT", [P, SHP], F32, kind="ExternalOutput")

    # ---- collective tables ----
    cc_in = [nc.dram_tensor(f"cc{i}_in", [SHP, H], BF16, kind="Internal")
             for i in (1, 2)]
    cc_out = [nc.dram_tensor(f"cc{i}_out", [NC * SHP, H], BF16,
                             kind="Internal", addr_space="Shared")
              for i in (1, 2)]

    rg = [list(range(NC))]

    with tile.TileContext(nc) as tc:
        with (
            tc.tile_pool(name="const", bufs=1) as cpool,
            tc.tile_pool(name="resident", bufs=1) as rpool,
            ExitStack() as mstack,
        ):
            # ---- constants ----
            ident = cpool.tile([P, P], BF16)
            make_identity(nc, ident[:])
            bias_t = cpool.tile([P, 9], F32)
            nc.sync.dma_start(out=bias_t[:], in_=biases[:])

            wt = {}
            for nm, kb in [("wdes", KD), ("wtweet", KT), ("wnum", 1),
                           ("wcat", 1), ("win", 4), ("root1", 1),
                           ("rel10", 1), ("rel11", 1), ("root2", 1),
                           ("rel20", 1), ("rel21", 1), ("wcls", 1)]:
                t = cpool.tile([P, kb, H], BF16, tag=f"w_{nm}", name=f"w_{nm}")
                nc.sync.dma_start(
                    out=t[:], in_=wts[nm].rearrange("(k p) h -> p k h", p=P))
                wt[nm] = t

            # resident activations (transposed, [H, SHP] bf16)
            hT = [rpool.tile([P, SHP], BF16, tag="ht", name=f"hT{i}", bufs=2)
                  for i in range(2)]

            # =============== feature transform ===============
            fstack = ExitStack()
            fpool = fstack.enter_context(tc.tile_pool(name="featsb", bufs=2))
            fpp = fstack.enter_context(
                tc.tile_pool(name="featps", bufs=2, space="PSUM"))
            ntiles = (SHP + NTF - 1) // NTF
            for t in range(ntiles):
                n0 = t * NTF
                n1 = min(SHP, n0 + NTF)
                nn = n1 - n0
                xt = fpool.tile([P, KX, NTF], BF16, tag="xt", name="xt")
                nc.sync.dma_start(
                    out=xt[:, :, :nn],
                    in_=xT.rearrange("(k p) n -> p k n", p=P)[:, :, n0:n1])

                zb = []
                for bi, (wnm, ks, kn) in enumerate([
                        ("wdes", 0, KD), ("wtweet", KD, KT),
                        ("wnum", KD + KT, 1), ("wcat", KD + KT + 1, 1)]):
                    pz = fpp.tile([P, NTF], F32, tag=f"pz{bi}", name=f"pz{bi}", space="PSUM", bufs=1)
                    for k in range(kn):
                        nc.tensor.matmul(
                            out=pz[:, :nn], lhsT=wt[wnm][:, k, :],
                            rhs=xt[:, ks + k, :nn],
                            start=(k == 0), stop=(k == kn - 1))
                    v = fpool.tile([P, NTF], BF16, tag=f"v{bi}", name=f"v{bi}")
                    nc.scalar.activation(
                        out=v[:, :nn], in_=pz[:, :nn],
                        func=mybir.ActivationFunctionType.Identity,
                        bias=bias_t[:, bi:bi + 1])
                    z = fpool.tile([P, NTF], BF16, tag=f"z{bi}", name=f"z{bi}")
                    nc.vector.scalar_tensor_tensor(
                        out=z[:, :nn], in0=v[:, :nn], scalar=0.01,
                        in1=v[:, :nn], op0=mybir.AluOpType.mult,
                        op1=mybir.AluOpType.max)
                    zb.append(z)

                ph = fpp.tile([P, NTF], F32, tag="ph", name="ph", space="PSUM")
                for k in range(4):
                    nc.tensor.matmul(out=ph[:, :nn], lhsT=wt["win"][:, k, :],
                                     rhs=zb[k][:, :nn],
                                     start=(k == 0), stop=(k == 3))
                vh = fpool.tile([P, NTF], F32, tag="vh", name="vh")
                nc.scalar.activation(
                    out=vh[:, :nn], in_=ph[:, :nn],
                    func=mybir.ActivationFunctionType.Identity,
                    bias=bias_t[:, 4:5])
                nc.vector.scalar_tensor_tensor(
                    out=hT[0][:, n0:n1], in0=vh[:, :nn],
                    scalar=bias_t[:, 5:6], in1=vh[:, :nn],
                    op0=mybir.AluOpType.mult, op1=mybir.AluOpType.max)

            fstack.close()
            wpool = mstack.enter_context(tc.tile_pool(name="work", bufs=3))
            ppool = mstack.enter_context(
                tc.tile_pool(name="psum", bufs=2, space="PSUM"))

            # =============== per-layer helper ===============
            def emit_table(src_hT, cc_in_t, cc_out_t):
                RB = 7   # transpose blocks per staging tile / DMA
                ccv = cc_in_t.rearrange("(p t) h -> p t h", p=P)
                for b0 in range(0, TBLK, RB):
                    rows = wpool.tile([P, RB, P], BF16, tag="rows",
                                      name="rows", bufs=2)
                    for blk in range(b0, min(b0 + RB, TBLK)):
                        tp = ppool.tile([P, P], BF16, tag="tp", name="tp", space="PSUM", bufs=1)
                        nc.tensor.transpose(
                            out=tp[:], in_=src_hT[:, blk * P:(blk + 1) * P],
                            identity=ident[:])
                        nc.scalar.copy(out=rows[:, blk - b0, :], in_=tp[:])
                    nb = min(b0 + RB, TBLK) - b0
                    nc.sync.dma_start(
                        out=ccv[:, b0:b0 + nb, :], in_=rows[:, :nb, :])
                nc.gpsimd.collective_compute(
                    "AllGather", mybir.AluOpType.bypass,
                    ins=[cc_in_t[:]], outs=[cc_out_t[:]], replica_groups=rg)

            def emit_layer(li, h_in, table, rootw, relw, bias_col, finish):
                # per-stream gather state
                cur = {}
                stcur = [-1, None]   # st-stream chunk state

                def ensure_chunk(r, b, blkloc):
                    ch = blkloc // NBLK_CH
                    key = (r, b)
                    if cur.get(key, (-1,))[0] == ch:
                        return cur[key]
                    gblk0 = int(pl.stream_blk_base[r, b]) + ch * NBLK_CH
                    it = wpool.tile([P, CHS // 16], I16, tag=f"idx{r}{b}", name=f"idx{r}{b}", bufs=2)
                    nc.sync.dma_start(
                        out=it[:],
                        in_=idxt[:, gblk0 * P // 16:(gblk0 + NBLK_CH) * P // 16])
                    gt = wpool.tile([P, NBLK_CH, P], BF16, tag=f"st{r}{b}", name=f"st{r}{b}", bufs=3)
                    # final stream chunk: trailing -1 idxs are dropped by the
                    # Q7 kernel; pass the matching valid count
                    nvalid = min(CHS, int(pl.stream_vblk[r, b]
                                          - ch * NBLK_CH) * P)
                    nc.gpsimd.dma_gather(
                        out_ap=gt[:],
                        in_ap=table[b * BR:min((b + 1) * BR, TROWS), :],
                        idxs_ap=it[:], num_idxs=CHS, num_idxs_reg=nvalid,
                        elem_size=H, single_packet=False,
                        queue_num=(r * BANKS + b) % 4)
                    cur[key] = (ch, gt)
                    return cur[key]

                def ensure_st(m):
                    ch = m // ST_CH
                    if stcur[0] != ch:
                        sc = wpool.tile([P, ST_CH, WIN], BF16, tag="stc", name="stc", bufs=10)
                        nc.scalar.dma_start(
                            out=sc[:], in_=stt[:, ch * ST_CH:(ch + 1) * ST_CH, :])
                        stcur[0] = ch
                        stcur[1] = sc
                    return stcur[1]

                def emit_tail(w, agg):
                    # root+rel matmuls and output chain for window w; emitted
                    # one window late so PE never head-of-line blocks on the
                    # Scalar psum->sbuf copies.
                    ws = slice(w * WIN, (w + 1) * WIN)
                    po = ppool.tile([P, WIN], F32, tag="po", name="po", space="PSUM")
                    nc.tensor.matmul(out=po[:], lhsT=rootw[:, 0, :],
                                     rhs=h_in[:, ws], start=True, stop=False)
                    for r in range(R):
                        nc.tensor.matmul(out=po[:], lhsT=relw[r][:, 0, :],
                                         rhs=agg[r][:], start=False,
                                         stop=(r == R - 1))
                    finish(w, ws, po)

                m = 0
                prev = None
                for w in range(NW):
                    agg = []
                    for r in range(R):
                        pa = ppool.tile([P, WIN], F32, tag=f"agg{r}", name=f"agg{r}",
                                        space="PSUM")
                        lst = pl.emis_rw[(r, w)]
                        nblk_w = len(lst)
                        for j, (b, Bg) in enumerate(lst):
                            blkloc = Bg - int(pl.stream_blk_base[r, b])
                            ch, gt = ensure_chunk(r, b, blkloc)
                            pos = blkloc - ch * NBLK_CH
                            sc = ensure_st(m)
                            nc.tensor.matmul(
                                out=pa[:], lhsT=gt[:, pos, :],
                                rhs=sc[:, m % ST_CH, :],
                                start=(j == 0), stop=(j == nblk_w - 1))
                            m += 1
                        asb = wpool.tile([P, WIN], BF16, tag=f"asb{r}", name=f"asb{r}", bufs=2)
                        if nblk_w == 0:
                            nc.vector.memset(asb[:], 0.0)
                        else:
                            nc.scalar.copy(out=asb[:], in_=pa[:])
                        agg.append(asb)

                    if prev is not None:
                        emit_tail(prev[0], prev[1])
                    prev = (w, agg)
                emit_tail(prev[0], prev[1])
                assert m == pl.EMIT_TOT

            def finish_h(h_out, bias_col):
                def fn(w, ws, po):
                    nc.scalar.activation(
                        out=h_out[:, ws], in_=po[:],
                        func=mybir.ActivationFunctionType.Identity,
                        bias=bias_t[:, bias_col:bias_col + 1])
                return fn

            def finish_cls(bias_col):
                # layer-2 output -> classifier -> DRAM, fused per window
                def fn(w, ws, po):
                    h2 = wpool.tile([P, WIN], BF16, tag="h2", name="h2", bufs=2)
                    nc.scalar.activation(
                        out=h2[:], in_=po[:],
                        func=mybir.ActivationFunctionType.Identity,
                        bias=bias_t[:, bias_col:bias_col + 1])
                    pc = ppool.tile([P, WIN], F32, tag="pc", name="pc", space="PSUM", bufs=1)
                    nc.tensor.matmul(out=pc[:], lhsT=wt["wcls"][:, 0, :],
                                     rhs=h2[:], start=True, stop=True)
                    oc = wpool.tile([P, WIN], F32, tag="oc", name="oc", bufs=2)
                    nc.scalar.activation(
                        out=oc[:], in_=pc[:],
                        func=mybir.ActivationFunctionType.Identity,
                        bias=bias_t[:, 8:9])
                    nc.sync.dma_start(out=outT[:, ws], in_=oc[:])
                return fn

            # table of h0 + layer 1
            emit_table(hT[0], cc_in[0], cc_out[0])
            emit_layer(0, hT[0], cc_out[0],
                       wt["root1"], [wt["rel10"], wt["rel11"]], 6,
                       finish_h(hT[1], 6))
            # table of h1 + layer 2 (classifier fused)
            emit_table(hT[1], cc_in[1], cc_out[1])
            emit_layer(1, hT[1], cc_out[1],
                       wt["root2"], [wt["rel20"], wt["rel21"]], 7,
                       finish_cls(7))

    nc.compile()
    return nc


# ---------------------------------------------------------------------------
# entry point
# ---------------------------------------------------------------------------

def kernel(**inputs):
    cfg = _derived(CFG)
    return _kernel_impl(inputs, cfg)


def _kernel_impl(inputs, cfg, trace=False):
    d = cfg
    NC, SH, SHP = d["NC"], d["SH"], d["SHP"]

    pl = build_plan(inputs["edge_index"], inputs["edge_type"], d)
    xs = prep_x(np.asarray(inputs["x"], np.float32), d)
    w = prep_weights(inputs, d)

    nc = build_bass(d, pl)

    in_maps = []
    for c in range(NC):
        m = {"xT": xs[c], "idxt": pl.idx16[c], "stt": pl.stT[c],
             "biases": w["biases"]}
        for nm in ["wdes", "wtweet", "wnum", "wcat", "win", "root1", "rel10",
                   "rel11", "root2", "rel20", "rel21", "wcls"]:
            m[nm] = w[nm]
        in_maps.append(m)

    res = run_bass_kernel_spmd(nc, in_maps, core_ids=list(range(NC)),
                               trace=trace)

    out = np.empty((NC * SH, d["H"]), np.float32)
    for c in range(NC):
        out[c * SH:(c + 1) * SH] = res.results[c]["outT"].T[:SH]
    if trace:
        return out, res
    return out


# revision 23
# speedup vs baseline: 1.6691x; 1.3525x over previous
"""BotRGCN Trainium2 kernel: feature transform + 2 RGCN layers + classifier.

Sharding: nodes split across 8 cores by id (12500/core, padded to 12544).
Edges partitioned by destination shard; per (relation, dst-window, src-bank)
groups padded to a block structure uniform across cores so a single SPMD
program serves all 8 cores. Source features exchanged via bf16 AllGather of
the per-layer node-feature table; gathers via int16 dma_gather per src bank,
round-robined over 4 SWDGE queues so descriptor generation uses all Q7 core
pairs. The per-block scatter one-hot (edge -> dst column, mean weight folded
in) is precomputed on host and streamed from DRAM, keeping DVE off the
critical path.
"""

import sys

sys.path.insert(0, "/opt/trn_rl_repo")

from contextlib import ExitStack

import numpy as np
import ml_dtypes

import concourse.bass as bass
import concourse.bacc as bacc
import concourse.mybir as mybir
import concourse.tile as tile
from concourse.masks import make_identity
from concourse.bass_utils import run_bass_kernel_spmd

BF16 = mybir.dt.bfloat16
F32 = mybir.dt.float32
I16 = mybir.dt.int16

P = 128

# full-problem config (test.py overrides for mini runs)
CFG = dict(
    N=100000,        # nodes
    NC=8,            # cores
    R=2,             # relations
    H=128,
    DES=768, TWEET=768, NUMP=6, CATP=11,
    WIN=256,         # dst window (PSUM free dim)
    NBLK_CH=12,      # gather-chunk size in 128-edge blocks
    ST_CH=8,         # st-stream chunk size in blocks
    BANKROWS=25088,  # gather-table bank rows (< 2^15)
    NTF=512,         # feature-stage node tile
    ST_FP8=True,     # stream scatter tiles as fp8e4 (halves st DMA bytes)
)


def _derived(cfg):
    d = dict(cfg)
    d["SH"] = cfg["N"] // cfg["NC"]
    d["SHP"] = ((d["SH"] + P - 1) // P) * P
    d["NW"] = d["SHP"] // cfg["WIN"]
    assert d["SHP"] % cfg["WIN"] == 0
    d["TROWS"] = cfg["NC"] * d["SHP"]           # padded table rows
    d["BANKS"] = (d["TROWS"] + cfg["BANKROWS"] - 1) // cfg["BANKROWS"]
    d["TBLK"] = d["SHP"] // P                   # 128-row blobs per core
    # x feature layout: [des | tweet | num(pad to 128) | cat(pad to 128)]
    d["KDES"] = cfg["DES"] // P
    d["KTWEET"] = cfg["TWEET"] // P
    d["KX"] = d["KDES"] + d["KTWEET"] + 2
    d["XROWS"] = d["KX"] * P
    return d


# ---------------------------------------------------------------------------
# host-side graph planning
# ---------------------------------------------------------------------------

class Plan:
    pass


def build_plan(edge_index, edge_type, cfg):
    """Group edges per core by (rel, dst-window, src-bank); pad each group to a
    whole number of 128-edge blocks, uniform across cores. Returns per-core
    gather-index arrays, the streamed scatter one-hot tiles (emission order),
    plus the uniform block structure."""
    d = cfg
    NC, SH, SHP, WIN, NW = d["NC"], d["SH"], d["SHP"], d["WIN"], d["NW"]
    BANKS, BR, NBLK_CH = d["BANKS"], d["BANKROWS"], d["NBLK_CH"]
    ST_CH = d["ST_CH"]
    R = d["R"]
    N = d["N"]
    TBLK = d["TBLK"]

    src = np.asarray(edge_index[0], dtype=np.int64)
    dst = np.asarray(edge_index[1], dtype=np.int64)
    et = np.asarray(edge_type, dtype=np.int64)

    core = dst // SH
    dl = dst - core * SH
    # table row of a (padded) node: blob layout [p][t] per core
    sl = src - (src // SH) * SH
    ps = (src // SH) * SHP + (sl % P) * TBLK + (sl // P)
    bank = ps // BR
    bidx = (ps - bank * BR).astype(np.int16)
    win = dl // WIN
    dw = (dl - win * WIN).astype(np.int64)

    # per-(rel, node) in-degree -> per-edge mean weight
    cnt = np.bincount(et * N + dst, minlength=R * N).reshape(R, N)
    wv = (1.0 / np.maximum(cnt, 1.0))[et, dst].astype(np.float32)

    # group = (rel, bank, win); uniform EXACT sizes = max count over cores.
    # Groups are laid out back-to-back within each (rel, bank) stream (no
    # 128-alignment): a 128-slot block may span several groups; each
    # (group, block) pair becomes one matmul emission whose streamed st tile
    # is zero outside the group's slots.
    NG = R * BANKS * NW
    gid = (et * BANKS + bank) * NW + win
    counts = np.bincount(core * NG + gid, minlength=NC * NG).reshape(NC, NG)
    gsz = counts.max(axis=0).reshape(R, BANKS, NW).astype(np.int64)

    CHSL = NBLK_CH * P             # slots per gather chunk
    group_slot_base = np.zeros((R, BANKS, NW), np.int64)
    stream_blk_base = np.zeros((R, BANKS), np.int64)   # block units
    stream_nblk = np.zeros((R, BANKS), np.int64)       # incl. pad, blocks
    stream_vblk = np.zeros((R, BANKS), np.int64)       # gathered blocks
    base = 0                                           # slot units
    for r in range(R):
        for b in range(BANKS):
            stream_blk_base[r, b] = base // P
            v = 0
            for w in range(NW):
                group_slot_base[r, b, w] = base + v
                v += int(gsz[r, b, w])
            vpad = ((v + CHSL - 1) // CHSL) * CHSL
            if vpad == 0:
                vpad = CHSL
            stream_vblk[r, b] = (v + P - 1) // P
            stream_nblk[r, b] = vpad // P
            base += vpad
    TOTSLOT = base
    TOTBLK = TOTSLOT // P

    # emission list: m over (w, r, b, block-of-group)
    emis_rw = {}
    m_base = np.zeros((R, BANKS, NW), np.int64)
    first_blk = np.zeros((R, BANKS, NW), np.int64)
    m = 0
    for w in range(NW):
        for r in range(R):
            lst = []
            for b in range(BANKS):
                a = int(group_slot_base[r, b, w])
                e = a + int(gsz[r, b, w])
                if e > a:
                    B0, B1 = a // P, (e - 1) // P
                    m_base[r, b, w] = m
                    first_blk[r, b, w] = B0
                    for B in range(B0, B1 + 1):
                        lst.append((b, B))
                        m += 1
            emis_rw[(r, w)] = lst
    EMIT_TOT = m
    EMIT_PAD = ((EMIT_TOT + ST_CH - 1) // ST_CH) * ST_CH

    okey = core * NG + gid
    order = np.argsort(okey, kind="stable")
    so = okey[order]
    first_of = np.r_[True, so[1:] != so[:-1]]
    idx_in_run = np.arange(len(so)) - np.maximum.accumulate(
        np.where(first_of, np.arange(len(so)), 0)
    )
    gsb_flat = group_slot_base.transpose(0, 1, 2).reshape(-1)  # [r,b,w] order
    # gid is (r*BANKS+b)*NW+w which matches the flat [r,b,w] layout
    slot = gsb_flat[so % NG] + idx_in_run

    idx16 = np.zeros((NC, 8 * 16, TOTSLOT // 16), np.int16)
    # slots beyond each stream's gathered region: -1 (trailing, trimmed)
    for r in range(R):
        for b in range(BANKS):
            p0 = (stream_blk_base[r, b] + stream_vblk[r, b]) * P
            p1 = (stream_blk_base[r, b] + stream_nblk[r, b]) * P
            if p1 > p0:
                s = np.arange(p0, p1)
                for g in range(8):
                    idx16[:, 16 * g + (s % 16), s // 16] = -1

    ecore = core[order]
    col = slot // 16
    prow = (slot % 16).astype(np.int64)
    for g in range(8):
        idx16[ecore, 16 * g + prow, col] = bidx[order]

    # streamed scatter tiles, emission order: stT[p, m, j].
    # fp8 mode: pure 0/1 one-hot (exact in e4m3); the mean weight is applied
    # per dst column via the separate cntT stream. bf16 mode folds it in.
    mb_flat = m_base.reshape(-1)
    fb_flat = first_blk.reshape(-1)
    m_edge = mb_flat[so % NG] + (slot // P - fb_flat[so % NG])
    if d.get("ST_FP8"):
        stT = np.zeros((NC, P, EMIT_PAD, WIN), ml_dtypes.float8_e4m3)
        stT[ecore, slot % P, m_edge, dw[order]] = 1.0
    else:
        stT = np.zeros((NC, P, EMIT_PAD, WIN), ml_dtypes.bfloat16)
        stT[ecore, slot % P, m_edge, dw[order]] = wv[order]
    # per-(rel, window) replicated column scales [P, WIN] (per core)
    civ = (1.0 / np.maximum(cnt, 1.0))  # [R, N]
    cnt_t = np.zeros((NC, P, R * NW, WIN), ml_dtypes.bfloat16)
    for c in range(NC):
        loc = np.zeros((R, SHP), np.float32)
        loc[:, :SH] = civ[:, c * SH:(c + 1) * SH]
        cnt_t[c] = np.broadcast_to(
            loc.reshape(1, R * NW, WIN), (P, R * NW, WIN)).astype(
                ml_dtypes.bfloat16)

    pl = Plan()
    pl.idx16 = idx16.reshape(NC, P, TOTSLOT // 16)
    pl.stT = stT
    pl.cnt_t = cnt_t
    pl.emis_rw = emis_rw
    pl.TOTBLK = TOTBLK
    pl.EMIT_TOT = EMIT_TOT
    pl.EMIT_PAD = EMIT_PAD
    pl.stream_blk_base = stream_blk_base
    pl.stream_nblk = stream_nblk
    pl.stream_vblk = stream_vblk
    return pl


def prep_x(x, cfg):
    """Per-core transposed bf16 feature blocks [XROWS, SHP]."""
    d = cfg
    NC, SH, SHP = d["NC"], d["SH"], d["SHP"]
    NUMP, TWEET, CATP, DES = d["NUMP"], d["TWEET"], d["CATP"], d["DES"]
    KD, KT = d["KDES"], d["KTWEET"]
    out = np.zeros((NC, d["XROWS"], SHP), ml_dtypes.bfloat16)
    for c in range(NC):
        xs = x[c * SH:(c + 1) * SH]
        xT = np.zeros((d["XROWS"], SHP), np.float32)
        xT[:DES, :SH] = xs[:, NUMP + TWEET + CATP:].T
        xT[DES:DES + TWEET, :SH] = xs[:, NUMP:NUMP + TWEET].T
        xT[(KD + KT) * P:(KD + KT) * P + NUMP, :SH] = xs[:, :NUMP].T
        xT[(KD + KT + 1) * P:(KD + KT + 1) * P + CATP, :SH] = \
            xs[:, NUMP + TWEET:NUMP + TWEET + CATP].T
        out[c] = xT.astype(ml_dtypes.bfloat16)
    return out


def prep_weights(inp, cfg):
    """bf16 weight blocks + packed fp32 biases."""
    bf = lambda a: np.asarray(a, np.float32).astype(ml_dtypes.bfloat16)
    d = cfg
    wnum = np.zeros((P, d["H"]), np.float32)
    wnum[:d["NUMP"]] = inp["W_num"]
    wcat = np.zeros((P, d["H"]), np.float32)
    wcat[:d["CATP"]] = inp["W_cat"]
    w = {
        "wdes": bf(inp["W_des"]), "wtweet": bf(inp["W_tweet"]),
        "wnum": bf(wnum), "wcat": bf(wcat), "win": bf(inp["W_in"]),
        "root1": bf(inp["root1"]), "rel10": bf(inp["rel1"][0]),
        "rel11": bf(inp["rel1"][1]),
        "root2": bf(inp["root2"]), "rel20": bf(inp["rel2"][0]),
        "rel21": bf(inp["rel2"][1]), "wcls": bf(inp["W_cls"]),
    }
    biases = np.stack(
        [inp["b_des"], inp["b_tweet"], inp["b_num"], inp["b_cat"],
         inp["b_in"], inp["prelu_a"], inp["bias1"], inp["bias2"],
         inp["b_cls"]], axis=1).astype(np.float32)   # [128, 9]
    w["biases"] = biases
    return w


# ---------------------------------------------------------------------------
# bass program
# ---------------------------------------------------------------------------

def build_bass(cfg, pl):
    d = cfg
    NC, SHP, WIN, NW, NTF = d["NC"], d["SHP"], d["WIN"], d["NW"], d["NTF"]
    BANKS, BR, NBLK_CH = d["BANKS"], d["BANKROWS"], d["NBLK_CH"]
    ST_CH = d["ST_CH"]
    R, H = d["R"], d["H"]
    KD, KT, KX = d["KDES"], d["KTWEET"], d["KX"]
    TBLK = d["TBLK"]
    TROWS = d["TROWS"]
    CHS = NBLK_CH * P      # idx slots per chunk

    nc = bacc.Bacc(None, target_bir_lowering=False, debug=False,
                   num_devices=NC, num_swdge_queues=4,
                   dynamic_dma_scratch_size=32768)

    # ---- I/O ----
    xT = nc.dram_tensor("xT", [d["XROWS"], SHP], BF16, kind="ExternalInput")
    idxt = nc.dram_tensor("idxt", [P, pl.TOTBLK * P // 16], I16, kind="ExternalInput")
    STDT = mybir.dt.float8e4 if d.get("ST_FP8") else BF16
    stt = nc.dram_tensor("stt", [P, pl.EMIT_PAD, WIN], STDT, kind="ExternalInput")
    wts = {}
    for nm, shp in [("wdes", [d["DES"], H]), ("wtweet", [d["TWEET"], H]),
                    ("wnum", [P, H]), ("wcat", [P, H]), ("win", [4 * P, H]),
                    ("root1", [H, H]), ("rel10", [H, H]), ("rel11", [H, H]),
                    ("root2", [H, H]), ("rel20", [H, H]), ("rel21", [H, H]),
                    ("wcls", [H, H])]:
        wts[nm] = nc.dram_tensor(nm, shp, BF16, kind="ExternalInput")
    biases = nc.dram_tensor("biases", [P, 9], F32, kind="ExternalInput")
    cntt = nc.dram_tensor("cntt", [P, R * NW, WIN], BF16, kind="ExternalInput")
    outT = nc.dram_tensor("outT", [P, SHP], F32, kind="ExternalOutput")

    # ---- collective tables ----
    cc_in = [nc.dram_tensor(f"cc{i}_in", [SHP, H], BF16, kind="Internal")
             for i in (1, 2)]
    cc_out = [nc.dram_tensor(f"cc{i}_out", [NC * SHP, H], BF16,
                             kind="Internal", addr_space="Shared")
              for i in (1, 2)]

    rg = [list(range(NC))]

    with tile.TileContext(nc) as tc:
        with (
            tc.tile_pool(name="const", bufs=1) as cpool,
            tc.tile_pool(name="resident", bufs=1) as rpool,
            ExitStack() as mstack,
        ):
            # ---- constants ----
            ident = cpool.tile([P, P], BF16)
            make_identity(nc, ident[:])
            bias_t = cpool.tile([P, 9], F32)
            nc.sync.dma_start(out=bias_t[:], in_=biases[:])

            wt = {}
            for nm, kb in [("wdes", KD), ("wtweet", KT), ("wnum", 1),
                           ("wcat", 1), ("win", 4), ("root1", 1),
                           ("rel10", 1), ("rel11", 1), ("root2", 1),
                           ("rel20", 1), ("rel21", 1), ("wcls", 1)]:
                t = cpool.tile([P, kb, H], BF16, tag=f"w_{nm}", name=f"w_{nm}")
                nc.sync.dma_start(
                    out=t[:], in_=wts[nm].rearrange("(k p) h -> p k h", p=P))
                wt[nm] = t

            # resident activations (transposed, [H, SHP] bf16)
            hT = [rpool.tile([P, SHP], BF16, tag="ht", name=f"hT{i}", bufs=2)
                  for i in range(2)]
            ccv = [t.rearrange("(p t) h -> p t h", p=P) for t in cc_in]

            def emit_rows(src_hT, cc_i, b0, nb):
                # transpose blobs [b0, b0+nb) of src_hT into the collective
                # input table; interleaved with the producer loop
                rows = rpool.tile([P, 4, P], BF16, tag="rows",
                                  name="rows", bufs=2)
                for k in range(nb):
                    tp = tppool.tile([P, P], BF16, tag="tp", name="tp",
                                     space="PSUM", bufs=1)
                    nc.tensor.transpose(
                        out=tp[:], in_=src_hT[:, (b0 + k) * P:(b0 + k + 1) * P],
                        identity=ident[:])
                    nc.scalar.copy(out=rows[:, k, :], in_=tp[:])
                nc.sync.dma_start(
                    out=ccv[cc_i][:, b0:b0 + nb, :], in_=rows[:, :nb, :])

            wpool = mstack.enter_context(tc.tile_pool(name="work", bufs=3))
            tppool = mstack.enter_context(
                tc.tile_pool(name="tpps", bufs=1, space="PSUM"))

            # =============== feature transform ===============
            fstack = ExitStack()
            fpool = fstack.enter_context(tc.tile_pool(name="featsb", bufs=2))
            fpp = fstack.enter_context(
                tc.tile_pool(name="featps", bufs=2, space="PSUM"))
            ntiles = (SHP + NTF - 1) // NTF
            for t in range(ntiles):
                n0 = t * NTF
                n1 = min(SHP, n0 + NTF)
                nn = n1 - n0
                xt = fpool.tile([P, KX, NTF], BF16, tag="xt", name="xt")
                nc.sync.dma_start(
                    out=xt[:, :, :nn],
                    in_=xT.rearrange("(k p) n -> p k n", p=P)[:, :, n0:n1])

                zb = []
                for bi, (wnm, ks, kn) in enumerate([
                        ("wdes", 0, KD), ("wtweet", KD, KT),
                        ("wnum", KD + KT, 1), ("wcat", KD + KT + 1, 1)]):
                    pz = fpp.tile([P, NTF], F32, tag=f"pz{bi}", name=f"pz{bi}", space="PSUM", bufs=1)
                    for k in range(kn):
                        nc.tensor.matmul(
                            out=pz[:, :nn], lhsT=wt[wnm][:, k, :],
                            rhs=xt[:, ks + k, :nn],
                            start=(k == 0), stop=(k == kn - 1))
                    v = fpool.tile([P, NTF], BF16, tag=f"v{bi}", name=f"v{bi}")
                    nc.scalar.activation(
                        out=v[:, :nn], in_=pz[:, :nn],
                        func=mybir.ActivationFunctionType.Identity,
                        bias=bias_t[:, bi:bi + 1])
                    z = fpool.tile([P, NTF], BF16, tag=f"z{bi}", name=f"z{bi}")
                    nc.vector.scalar_tensor_tensor(
                        out=z[:, :nn], in0=v[:, :nn], scalar=0.01,
                        in1=v[:, :nn], op0=mybir.AluOpType.mult,
                        op1=mybir.AluOpType.max)
                    zb.append(z)

                ph = fpp.tile([P, NTF], F32, tag="ph", name="ph", space="PSUM")
                for k in range(4):
                    nc.tensor.matmul(out=ph[:, :nn], lhsT=wt["win"][:, k, :],
                                     rhs=zb[k][:, :nn],
                                     start=(k == 0), stop=(k == 3))
                vh = fpool.tile([P, NTF], F32, tag="vh", name="vh")
                nc.scalar.activation(
                    out=vh[:, :nn], in_=ph[:, :nn],
                    func=mybir.ActivationFunctionType.Identity,
                    bias=bias_t[:, 4:5])
                nc.vector.scalar_tensor_tensor(
                    out=hT[0][:, n0:n1], in0=vh[:, :nn],
                    scalar=bias_t[:, 5:6], in1=vh[:, :nn],
                    op0=mybir.AluOpType.mult, op1=mybir.AluOpType.max)
                for b0 in range(n0 // P, (n1 + P - 1) // P, 4):
                    emit_rows(hT[0], 0, b0, min(4, (n1 + P - 1) // P - b0))

            fstack.close()
            ppool = mstack.enter_context(
                tc.tile_pool(name="psum", bufs=2, space="PSUM"))

            # =============== per-layer helper ===============
            def emit_collective(i):
                nc.gpsimd.collective_compute(
                    "AllGather", mybir.AluOpType.bypass,
                    ins=[cc_in[i][:]], outs=[cc_out[i][:]], replica_groups=rg)

            def emit_layer(li, h_in, table, rootw, relw, bias_col, finish):
                # per-stream gather state
                cur = {}
                stcur = [-1, None]   # st-stream chunk state

                def ensure_chunk(r, b, blkloc):
                    ch = blkloc // NBLK_CH
                    key = (r, b)
                    if cur.get(key, (-1,))[0] == ch:
                        return cur[key]
                    gblk0 = int(pl.stream_blk_base[r, b]) + ch * NBLK_CH
                    it = wpool.tile([P, CHS // 16], I16, tag=f"idx{r}{b}", name=f"idx{r}{b}", bufs=2)
                    nc.sync.dma_start(
                        out=it[:],
                        in_=idxt[:, gblk0 * P // 16:(gblk0 + NBLK_CH) * P // 16])
                    gt = wpool.tile([P, NBLK_CH, P], BF16, tag=f"st{r}{b}", name=f"st{r}{b}", bufs=3)
                    # final stream chunk: trailing -1 idxs are dropped by the
                    # Q7 kernel; pass the matching valid count
                    nvalid = min(CHS, int(pl.stream_vblk[r, b]
                                          - ch * NBLK_CH) * P)
                    nc.gpsimd.dma_gather(
                        out_ap=gt[:],
                        in_ap=table[b * BR:min((b + 1) * BR, TROWS), :],
                        idxs_ap=it[:], num_idxs=CHS, num_idxs_reg=nvalid,
                        elem_size=H, single_packet=False,
                        queue_num=(r * BANKS + b) % 4)
                    cur[key] = (ch, gt)
                    return cur[key]

                def ensure_st(m):
                    ch = m // ST_CH
                    if stcur[0] != ch:
                        sc = wpool.tile([P, ST_CH, WIN], STDT, tag="stc", name="stc", bufs=10)
                        nc.scalar.dma_start(
                            out=sc[:], in_=stt[:, ch * ST_CH:(ch + 1) * ST_CH, :])
                        stcur[0] = ch
                        stcur[1] = sc
                    return stcur[1]

                def emit_tail(w, agg):
                    # root+rel matmuls and output chain for window w; emitted
                    # one window late so PE never head-of-line blocks on the
                    # Scalar psum->sbuf copies.
                    ws = slice(w * WIN, (w + 1) * WIN)
                    po = ppool.tile([P, WIN], F32, tag="po", name="po", space="PSUM")
                    nc.tensor.matmul(out=po[:], lhsT=rootw[:, 0, :],
                                     rhs=h_in[:, ws], start=True, stop=False)
                    for r in range(R):
                        nc.tensor.matmul(out=po[:], lhsT=relw[r][:, 0, :],
                                         rhs=agg[r][:], start=False,
                                         stop=(r == R - 1))
                    finish(w, ws, po)

                m = 0
                prev = None
                for w in range(NW):
                    agg = []
                    for r in range(R):
                        ct = wpool.tile([P, WIN], BF16, tag="cnt", name="ct", bufs=4)
                        nc.sync.dma_start(out=ct[:], in_=cntt[:, r * NW + w, :])
                        pa = ppool.tile([P, WIN], F32, tag=f"agg{r}", name=f"agg{r}",
                                        space="PSUM")
                        lst = pl.emis_rw[(r, w)]
                        nblk_w = len(lst)
                        for j, (b, Bg) in enumerate(lst):
                            blkloc = Bg - int(pl.stream_blk_base[r, b])
                            ch, gt = ensure_chunk(r, b, blkloc)
                            pos = blkloc - ch * NBLK_CH
                            sc = ensure_st(m)
                            nc.tensor.matmul(
                                out=pa[:], lhsT=gt[:, pos, :],
                                rhs=sc[:, m % ST_CH, :],
                                start=(j == 0), stop=(j == nblk_w - 1))
                            m += 1
                        asb = wpool.tile([P, WIN], BF16, tag=f"asb{r}", name=f"asb{r}", bufs=2)
                        if nblk_w == 0:
                            nc.vector.memset(asb[:], 0.0)
                        else:
                            # apply per-dst mean weight while moving PSUM->SBUF
                            nc.vector.tensor_tensor(
                                out=asb[:], in0=pa[:], in1=ct[:],
                                op=mybir.AluOpType.mult)
                        agg.append(asb)

                    if prev is not None:
                        emit_tail(prev[0], prev[1])
                    prev = (w, agg)
                emit_tail(prev[0], prev[1])
                assert m == pl.EMIT_TOT

            def finish_h(h_out, bias_col):
                BW = WIN // P
                def fn(w, ws, po):
                    nc.scalar.activation(
                        out=h_out[:, ws], in_=po[:],
                        func=mybir.ActivationFunctionType.Identity,
                        bias=bias_t[:, bias_col:bias_col + 1])
                    emit_rows(h_out, 1, w * BW, BW)
                return fn

            def finish_cls(bias_col):
                # layer-2 output -> classifier -> DRAM, fused per window
                def fn(w, ws, po):
                    h2 = wpool.tile([P, WIN], BF16, tag="h2", name="h2", bufs=2)
                    nc.scalar.activation(
                        out=h2[:], in_=po[:],
                        func=mybir.ActivationFunctionType.Identity,
                        bias=bias_t[:, bias_col:bias_col + 1])
                    pc = ppool.tile([P, WIN], F32, tag="pc", name="pc", space="PSUM", bufs=1)
                    nc.tensor.matmul(out=pc[:], lhsT=wt["wcls"][:, 0, :],
                                     rhs=h2[:], start=True, stop=True)
                    oc = wpool.tile([P, WIN], F32, tag="oc", name="oc", bufs=2)
                    nc.scalar.activation(
                        out=oc[:], in_=pc[:],
                        func=mybir.ActivationFunctionType.Identity,
                        bias=bias_t[:, 8:9])
                    nc.sync.dma_start(out=outT[:, ws], in_=oc[:])
                return fn

            # table of h0 + layer 1 (rows were emitted by the feature loop)
            emit_collective(0)
            emit_layer(0, hT[0], cc_out[0],
                       wt["root1"], [wt["rel10"], wt["rel11"]], 6,
                       finish_h(hT[1], 6))
            # table of h1 + layer 2 (rows emitted by layer-1 finish)
            emit_collective(1)
            emit_layer(1, hT[1], cc_out[1],
                       wt["root2"], [wt["rel20"], wt["rel21"]], 7,
                       finish_cls(7))

    nc.compile()
    return nc


# ---------------------------------------------------------------------------
# entry point
# ---------------------------------------------------------------------------

def kernel(**inputs):
    cfg = _derived(CFG)
    return _kernel_impl(inputs, cfg)


def _kernel_impl(inputs, cfg, trace=False):
    d = cfg
    NC, SH, SHP = d["NC"], d["SH"], d["SHP"]

    pl = build_plan(inputs["edge_index"], inputs["edge_type"], d)
    xs = prep_x(np.asarray(inputs["x"], np.float32), d)
    w = prep_weights(inputs, d)

    nc = build_bass(d, pl)

    in_maps = []
    for c in range(NC):
        m = {"xT": xs[c], "idxt": pl.idx16[c], "stt": pl.stT[c],
             "cntt": pl.cnt_t[c],
             "biases": w["biases"]}
        for nm in ["wdes", "wtweet", "wnum", "wcat", "win", "root1", "rel10",
                   "rel11", "root2", "rel20", "rel21", "wcls"]:
            m[nm] = w[nm]
        in_maps.append(m)

    res = run_bass_kernel_spmd(nc, in_maps, core_ids=list(range(NC)),
                               trace=trace)

    out = np.empty((NC * SH, d["H"]), np.float32)
    for c in range(NC):
        out[c * SH:(c + 1) * SH] = res.results[c]["outT"].T[:SH]
    if trace:
        return out, res
    return out
